# revision 58
# baseline (speedup 1.0000x reference)
"""Additive-attention fused kernel for one TRN2 chip (8 NeuronCores).

Math (per batch b):
    q = queries @ W_q.T                       [Q, H]
    k = keys    @ W_k.T                       [K, H]
    scores[q,k] = sum_h w_v[h] * tanh(q[q,h] + k[k,h])
    attn = masked_softmax(scores, valid_len)  (mask: k >= L -> weight 0)
    out  = attn @ values                      [Q, V]

Sharding: data-parallel over batch B=4 x 2-way split of Q -> 8 cores,
each core handles [QH=512, :] of one batch.  No collectives needed.

v3 algorithm (v2 was a per-m DVE range-reduction + ACT Sin pipeline):

1. Sparse K: only kb = ceil(max(valid_lens)/128) k-blocks are live; the
   masked tail contributes exactly 0 through the pre-masked [values|1]
   operand, so keys/values beyond kb*128 are never shipped or touched.
2. tanh(z) ~ sum_m c_m sin(pi m z / FL) (odd Fourier series, weighted
   LSQ fit, M=8, |z| <= 2*BCLAMP), which makes scores ONE accumulated
   matmul over contraction (h, m, {sin,cos}).
3. Chebyshev recurrence for the features: with theta = 2 pi nu1 x,
       t_m = sin(m theta + phase),  t_{m+1} = 2 cos(theta) t_m - t_{m-1}
   so only m=1 needs ACT Sin (|arg| <= ~pi, where the table is exact);
   every further frequency is 2 cheap DVE ops (scalar_tensor_tensor +
   tensor_tensor) instead of range-reduction chains + Sin.
4. Hybrid fp8: frequencies m>=3 (small |c_m|) are quantized to fp8e4
   (q side pre-scaled x64) and their score contributions run as
   DoubleRow matmuls (2x PE throughput, contraction 256/instr); m=1,2
   stay fp16.  All q scales carry x64 so PSUM holds 64*scores; the exp
   reads PSUM with scale=1/64, bias=-2 (softmax shift for fp8 headroom
   elsewhere and smaller exp range).
5. Tail pipelining: score PSUM banks are split in two groups; exp of
   group A runs while PE still accumulates group B, then attn@V
   (fp16: fp8 weights/values fail the accuracy budget) accumulates
   per 128-query block with the denominator in an appended ones
   column; DVE reciprocal + gpsimd scale produce the fp16 output.

Engine budget per core (kb=6): DVE ~10us (recurrence), gpsimd ~9us
(scales/quantize/epilogue), PE ~9us (proj + scores + attn@V), ACT ~7us
(seeds + exp); HBM ~1.3MB.
"""

import math

import numpy as np

B, QFULL, KK = 4, 1024, 1024
D, H, V = 256, 64, 256
QH = 512            # Q rows per core
NCORES = 8

FM = 8              # number of frequencies
FL = 9.0            # half-period (= 2*BCLAMP so the m=1 seed args fit the
                    # Sin table domain [-pi, pi] exactly)
BCLAMP = 4.5        # clamp q/k projections to +-BCLAMP
MLOW = 5            # 3-level per-bank frequency stagger: banks 0-1 use FM,
                    # banks 2-3 use FM-1, banks 4+ use MLOW -- so all exps
                    # except the last pair's hide under the recurrence
QSCALE = 64.0       # q-feature prescale (PSUM holds QSCALE*scores)
EXP_BIAS = -2.0     # softmax shift


def _bank_M(kb):
    if kb <= 4:
        return [FM] * kb
    return [FM if j < 2 else (FM - 1 if j < 4 else MLOW) for j in range(kb)]

_STATE = {}


def _fit_coeffs():
    z = np.linspace(-2 * BCLAMP, 2 * BCLAMP, 4001)
    w = np.exp(-z ** 2 / (2 * 2.2)) + 1e-4
    A = np.sin(np.pi / FL * np.outer(z, np.arange(1, FM + 1)))
    c = np.linalg.lstsq(A * np.sqrt(w)[:, None], np.tanh(z) * np.sqrt(w),
                        rcond=None)[0]
    return c.astype(np.float32)


COEFFS = _fit_coeffs()


def _fit_cospoly():
    """Even polynomial (deg 2 in x^2) for 2*cos(2 pi nu1 x), |x| <= BCLAMP."""
    x = np.linspace(-BCLAMP, BCLAMP, 2001)
    z = x * x
    A = np.stack([np.ones_like(z), z, z * z], axis=1)
    c = np.linalg.lstsq(A, 2.0 * np.cos(np.pi / FL * x), rcond=None)[0]
    return c.astype(np.float64)


COSP = _fit_cospoly()


def _build_nc(kb, n_iters=1):
    import contextlib
    import concourse.tile as tile
    from concourse import bacc, mybir

    F32 = mybir.dt.float32
    F16 = mybir.dt.float16
    F8 = mybir.dt.float8e4
    Sin = mybir.ActivationFunctionType.Sin
    Exp = mybir.ActivationFunctionType.Exp
    AOp = mybir.AluOpType
    DR = mybir.MatmulPerfMode.DoubleRow
    TWO_PI = 2.0 * math.pi

    KBC = kb * 128                  # live key columns
    W = QH + KBC                    # feature width (q cols | k cols)
    NU1 = 1.0 / (2.0 * FL)
    BANK_M = _bank_M(kb)
    if kb <= 4:
        GROUPS = [(j0, min(j0 + 2, kb)) for j0 in range(0, kb, 2)]
    else:
        GROUPS = [(0, 2), (2, 4), (4, kb)]
    NGRP = len(GROUPS)

    nc = bacc.Bacc()
    # host layouts are partition-major so each tensor is ONE DMA:
    # qT [128, 2, QH]: [p, c, :] = queries.T fp16 rows c*128+p
    # wpk [128, 2, 256]: [p, c, 0:128] = wqT2 rows, [.., 128:256] = wkT2
    #   (w*T2 = [D, 128] with the 64 output rows duplicated -> projections
    #    emit all 128 feature partitions directly, no SBUF dup pass)
    qT_d = nc.declare_dram_parameter("queriesT", [128, 2, QH], F16, isOutput=False)
    kT_d = nc.declare_dram_parameter("keysT", [128, 2, KBC], F16, isOutput=False)
    va_d = nc.declare_dram_parameter("vaug", [128, kb, V + 1], F16, isOutput=False)
    wpk_d = nc.declare_dram_parameter("wpk", [128, 2, 256], F16, isOutput=False)
    wvc_d = nc.declare_dram_parameter("wvc", [128, FM], F32, isOutput=False)
    out_d = nc.declare_dram_parameter("out", [QH, V], F16, isOutput=True)

    with tile.TileContext(nc) as tc:
        with (
            tc.tile_pool(name="singles", bufs=1) as singles,
            tc.tile_pool(name="tpool", bufs=5) as tpool,
            tc.tile_pool(name="upool", bufs=2) as upool,
            tc.tile_pool(name="qk8", bufs=1) as qk8,
            tc.tile_pool(name="outp", bufs=2) as outp,
            # one PSUM pool; per-tag slots: score group g owns tag "scg{g}",
            # projections and attn@V accumulators reuse those banks via the
            # same tags (Tile serializes on the tag's reader/writer chain)
            tc.tile_pool(name="ps_big", bufs=1, space="PSUM") as ps_big,
            tc.For_i(0, n_iters, 1,
                     hint_engines=(mybir.EngineType.PE, mybir.EngineType.DVE,
                                   mybir.EngineType.Activation,
                                   mybir.EngineType.SP, mybir.EngineType.Pool),
                     staggered_reset=True)
            if n_iters > 1 else contextlib.nullcontext(),
        ):
            # -------- stage inputs (5 DMAs total, split across 2 queues) ----
            wpk = singles.tile([128, 2, 256], F16, tag="wpk", name="wpk")
            nc.sync.dma_start(wpk, wpk_d[:, :, :])
            qTt = singles.tile([128, 2, QH], F16, tag="qTt", name="qTt")
            nc.sync.dma_start(qTt, qT_d[:, :, :])
            kTt = singles.tile([128, 2, KBC], F16, tag="kTt", name="kTt")
            nc.sync.dma_start(kTt, kT_d[:, :, :])
            wvc_sb = singles.tile([128, FM], F32, tag="wvc", name="wvc")
            nc.sync.dma_start(wvc_sb, wvc_d[:, :])
            v_aug = singles.tile([128, kb, V + 1], F16, tag="vaug", name="vaug")
            nc.sync.dma_start(v_aug, va_d[:, :, :])

            # phase biases (radians) for the m=1 seeds
            bq = singles.tile([128, 1], F32, tag="bq", name="bq")
            nc.vector.memset(bq[0:H, :], 0.0)
            nc.vector.memset(bq[H:128, :], math.pi / 2)
            bk = singles.tile([128, 1], F32, tag="bk", name="bk")
            nc.vector.memset(bk[0:H, :], math.pi / 2)
            nc.vector.memset(bk[H:128, :], 0.0)
            bc = singles.tile([128, 1], F32, tag="bc", name="bc")
            nc.vector.memset(bc, math.pi / 2)
            be = singles.tile([128, 1], F32, tag="be", name="be")
            nc.vector.memset(be, EXP_BIAS)
            # t_0 per-partition constants (m=2 seed): q side [0;1], k side [1;0]
            t0q = singles.tile([128, 1], F32, tag="t0q", name="t0q")
            nc.vector.memset(t0q[0:H, :], 0.0)
            nc.vector.memset(t0q[H:128, :], 1.0)
            t0k = singles.tile([128, 1], F32, tag="t0k", name="t0k")
            nc.vector.memset(t0k[0:H, :], 1.0)
            nc.vector.memset(t0k[H:128, :], 0.0)
            # dummy Sin to prefetch the trig act table before xt is ready
            dum = singles.tile([128, 1], F32, tag="dum", name="dum")
            nc.scalar.activation(dum, bq[:, 0:1], Sin)

            # ------------- projections + clamp -> fp16 [128, cols] -------------
            # W rows are host-duplicated so the matmul emits all 128 feature
            # partitions; separate per-side tiles avoid false tile deps.
            # each matmul output must stay inside one 512-float PSUM bank.
            xtq = singles.tile([128, QH], F16, tag="xtq", name="xtq")
            xtk = singles.tile([128, KBC], F16, tag="xtk", name="xtk")
            nkc = -(-KBC // 512)
            psq = ps_big.tile([128, 512], F32, tag=f"scg{NGRP - 1}", name="psq")
            for c in range(2):
                nc.tensor.matmul(psq, wpk[:, c, 0:128], qTt[:, c, :],
                                 start=(c == 0), stop=(c == 1))
            nc.vector.tensor_scalar(xtq, psq, BCLAMP, -BCLAMP, AOp.min, AOp.max)
            psk = ps_big.tile([128, nkc * 512], F32, tag="scg0", name="psk")
            for ci in range(nkc):
                c0, cw = ci * 512, min(512, KBC - ci * 512)
                for c in range(2):
                    nc.tensor.matmul(psk[:, c0:c0 + cw], wpk[:, c, 128:256],
                                     kTt[:, c, c0:c0 + cw],
                                     start=(c == 0), stop=(c == 1))
            # (gpsimd cannot touch PSUM on hw; clamp-k goes to DVE)
            nc.vector.tensor_scalar(xtk, psk[:, 0:KBC], BCLAMP,
                                    -BCLAMP, AOp.min, AOp.max)

            # ------------- m=1 seeds + 2cos multiplier -------------
            # t1 = sin(2 pi nu1 x + phase) on ACT; the recurrence multiplier
            # c2d = 2 cos(2 pi nu1 x) comes from a fitted even polynomial in
            # x^2 on DVE (xt is h-duplicated, so all 128 rows are right) --
            # it is ready as soon as the clamps land, well before t1.
            t1 = singles.tile([128, W], F16, tag="t1", name="t1")
            nc.scalar.activation(t1[:, 0:QH], xtq, Sin,
                                 bias=bq[:, 0:1], scale=TWO_PI * NU1)
            nc.scalar.activation(t1[:, QH:W], xtk, Sin,
                                 bias=bk[:, 0:1], scale=TWO_PI * NU1)
            c2d = singles.tile([128, W], F16, tag="c2d", name="c2d")
            zz = singles.tile([128, W], F16, tag="zz", name="zz")
            aa = singles.tile([128, W], F16, tag="aa", name="aa")
            bb = singles.tile([128, W], F16, tag="bb", name="bb")
            for x_in, s0, s1 in ((xtq, 0, QH), (xtk, QH, W)):
                z, a, b = zz[:, s0:s1], aa[:, s0:s1], bb[:, s0:s1]
                nc.vector.tensor_tensor(z, x_in, x_in, AOp.mult)
                nc.vector.tensor_scalar(a, z, float(COSP[2]), float(COSP[1]),
                                        AOp.mult, AOp.add)
                nc.vector.tensor_tensor(b, a, z, AOp.mult)
                nc.vector.tensor_scalar(c2d[:, s0:s1], b, float(COSP[0]),
                                        None, AOp.add)

            # ------------- features: Chebyshev recurrence + scales -------------
            # per m: u = t_{m-1} . c2d ; t_m = u - t_{m-2}   (DVE, all fp16)
            # plus qf_m = t_m[:, :QH] * (w_v c_m 64) for the score matmuls.
            # 3-level per-bank frequency stagger: each group's exp fires as
            # soon as its banks close, hiding under the rest of the recurrence
            bank_M = BANK_M
            qf16 = {}
            tms = {1: t1}
            for m in range(1, FM + 1):
                if m >= 2:
                    u = upool.tile([128, W], F16, tag="u", name=f"u{m}")
                    nc.vector.tensor_tensor(u, tms[m - 1], c2d, AOp.mult)
                    # every t_m is a PE matmul operand later; keep them all
                    tm = singles.tile([128, W], F16, tag=f"t{m}", name=f"t{m}")
                    if m == 2:
                        # t2 = u - t0 with t0 constant per partition-half
                        nc.vector.tensor_scalar(tm[:, 0:QH], u[:, 0:QH],
                                                t0q[:, 0:1], None, AOp.subtract)
                        nc.vector.tensor_scalar(tm[:, QH:W], u[:, QH:W],
                                                t0k[:, 0:1], None, AOp.subtract)
                    else:
                        nc.vector.tensor_tensor(tm, u, tms[m - 2], AOp.subtract)
                    tms[m] = tm
                tm = tms[m]
                qf = singles.tile([128, QH], F16, tag=f"qf{m}", name=f"qf{m}")
                nc.vector.tensor_scalar(qf, tm[:, 0:QH],
                                        wvc_sb[:, m - 1:m], None, AOp.mult)
                qf16[m] = qf

            # ------------- scores (PSUM accumulate), staggered tail -------------
            # banks grouped into per-tag PSUM tiles: full-M banks in pairs,
            # the LOWB low-M banks as the last group.  m-major emission =
            # matmuls fire the moment qf_m lands; low banks close at MLOW and
            # their exp hides under the rest of the recurrence.
            groups = GROUPS
            scg = []
            for g, (j0, j1) in enumerate(groups):
                scg.append(ps_big.tile([128, j1 - j0, QH], F32, tag=f"scg{g}",
                                       name=f"scg{g}"))
            jmap = {}
            group_M = []
            for g, (j0, j1) in enumerate(groups):
                for j in range(j0, j1):
                    jmap[j] = (g, j - j0)
                group_M.append(max(bank_M[j] for j in range(j0, j1)))
            p16 = [None] * NGRP
            for m in range(1, FM + 1):
                for j in range(kb):
                    if m > bank_M[j]:
                        continue
                    g, ji = jmap[j]
                    nc.tensor.matmul(
                        scg[g][:, ji, :],
                        tms[m][:, QH + j * 128:QH + (j + 1) * 128],
                        qf16[m], start=(m == 1), stop=(m == bank_M[j]))
                # emit each group's exp right after its last m-sweep so it
                # runs during the remaining recurrence
                for g, (j0, j1) in enumerate(groups):
                    if group_M[g] != m:
                        continue
                    pg = singles.tile([128, j1 - j0, QH], F16, tag=f"p16_{g}",
                                      name=f"p16_{g}")
                    nc.scalar.activation(pg.rearrange("p a b -> p (a b)"),
                                         scg[g].rearrange("p a b -> p (a b)"),
                                         Exp, bias=be[:, 0:1],
                                         scale=1.0 / QSCALE)
                    p16[g] = pg

            # ------------- attn @ [values | ones] + epilogue -------------
            # 4 concurrent accumulators: first pair in the last group's
            # banks (freed by the earliest exp), second in the mid group's;
            # j-major emission, latest-closing banks last
            av01 = ps_big.tile([128, 2, 512], F32, tag=f"scg{NGRP - 1}",
                               name="av01")
            av23 = ps_big.tile([128, 2, 512], F32,
                               tag=f"scg{1 if NGRP >= 2 else 0}", name="av23")
            avs = [av01[:, 0, 0:V + 1], av01[:, 1, 0:V + 1],
                   av23[:, 0, 0:V + 1], av23[:, 1, 0:V + 1]]
            jorder = sorted(range(kb), key=lambda j: (bank_M[j], -j))
            for idx, j in enumerate(jorder):
                g, ji = jmap[j]
                pj = p16[g][:, ji, :]
                for qb in range(QH // 128):
                    nc.tensor.matmul(avs[qb], pj[:, qb * 128:(qb + 1) * 128],
                                     v_aug[:, j, :], start=(idx == 0),
                                     stop=(idx == kb - 1))
            # output in two half-tiles so the first DMA overlaps the rest;
            # epilogue on DVE (reciprocal + scale), ACT is done after exps
            o16h = [outp.tile([128, 2, V], F16, tag=f"o16_{h}", name=f"o16_{h}")
                    for h in range(2)]
            for qb in range(QH // 128):
                av = avs[qb]
                rcp = outp.tile([128, 1], F32, tag="rcp", name=f"rcp{qb}")
                nc.vector.reciprocal(rcp, av[:, V:V + 1])
                nc.vector.tensor_scalar(o16h[qb // 2][:, qb % 2, :], av[:, 0:V],
                                        rcp[:, 0:1], None, AOp.mult)
                if qb % 2 == 1:
                    nc.sync.dma_start(
                        out_d.rearrange("(a p) v -> p a v", p=128)
                        [:, (qb - 1):(qb + 1), :], o16h[qb // 2])

    nc.finalize()
    return nc


def _build_runner(nc):
    """Cached multi-core PJRT runner (keeps the jitted callable so repeat
    calls don't retrace/recompile)."""
    import jax
    import numpy as _np
    from jax.sharding import Mesh, PartitionSpec
    from jax.experimental.shard_map import shard_map
    from concourse import bass2jax, mybir

    bass2jax.install_neuronx_cc_hook()

    partition_name = nc.partition_id_tensor.name if nc.partition_id_tensor else None
    in_names, out_names, out_avals, zero_outs = [], [], [], []
    for alloc in nc.m.functions[0].allocations:
        if not isinstance(alloc, mybir.MemoryLocationSet):
            continue
        name = alloc.memorylocations[0].name
        if alloc.kind == "ExternalInput":
            if name != partition_name:
                in_names.append(name)
        elif alloc.kind == "ExternalOutput":
            shape = tuple(alloc.tensor_shape)
            dtype = mybir.dt.np(alloc.dtype)
            out_names.append(name)
            out_avals.append(jax.core.ShapedArray(shape, dtype))
            zero_outs.append(_np.zeros(shape, dtype))
    n_params = len(in_names)
    n_outs = len(out_avals)
    all_in_names = list(in_names) + list(out_names)
    if partition_name is not None:
        all_in_names.append(partition_name)
    donate = tuple(range(n_params, n_params + n_outs))

    def _body(*args):
        operands = list(args)
        if partition_name is not None:
            operands.append(bass2jax.partition_id_tensor())
        outs = bass2jax._bass_exec_p.bind(
            *operands,
            out_avals=tuple(out_avals),
            in_names=tuple(all_in_names),
            out_names=tuple(out_names),
            lowering_input_output_aliases=(),
            sim_require_finite=True,
            sim_require_nnan=True,
            nc=nc,
        )
        return tuple(outs)

    devices = jax.devices()[:NCORES]
    assert len(devices) == NCORES, f"need {NCORES} cores, have {len(jax.devices())}"
    mesh = Mesh(_np.asarray(devices), ("core",))
    in_specs = (PartitionSpec("core"),) * (n_params + n_outs)
    out_specs = (PartitionSpec("core"),) * n_outs
    sharded = jax.jit(
        shard_map(_body, mesh=mesh, in_specs=in_specs, out_specs=out_specs,
                  check_rep=False),
        donate_argnums=donate, keep_unused=True)

    def run(in_maps):
        per_core = [[_np.asarray(m[name]) for name in in_names] for m in in_maps]
        concat_in = [
            _np.concatenate([per_core[c][i] for c in range(NCORES)], axis=0)
            for i in range(n_params)
        ]
        concat_zeros = [
            _np.zeros((NCORES * z.shape[0], *z.shape[1:]), z.dtype) for z in zero_outs
        ]
        out_arrs = sharded(*concat_in, *concat_zeros)
        return [
            {
                name: _np.asarray(out_arrs[i]).reshape(NCORES, *out_avals[i].shape)[c]
                for i, name in enumerate(out_names)
            }
            for c in range(NCORES)
        ]

    return run


def get_nc(n_iters=1, kb=None):
    if kb is None:
        kb = _STATE.get("kb", 6)
    key = f"nc{n_iters}_{kb}"
    if key not in _STATE:
        _STATE[key] = _build_nc(kb, n_iters)
    return _STATE[key]


def make_in_maps(queries, keys, values, valid_lens, W_q, W_k, w_v):
    queries = np.asarray(queries, dtype=np.float32)
    keys = np.asarray(keys, dtype=np.float32)
    values = np.asarray(values, dtype=np.float32)
    valid_lens = np.asarray(valid_lens)
    kb = max(1, min(KK // 128, int(-(-int(valid_lens.max()) // 128))))
    _STATE["kb"] = kb
    KBC = kb * 128
    # weights pack: W.T with output rows duplicated (128 feature rows),
    # chunked partition-major: wpk[p, c, 0:128] = wqT2 row c*128+p
    WqT2 = np.concatenate([np.asarray(W_q, np.float32).T] * 2, axis=1)  # [256,128]
    WkT2 = np.concatenate([np.asarray(W_k, np.float32).T] * 2, axis=1)
    wpk = np.concatenate([WqT2, WkT2], axis=1).astype(np.float16)      # [256,256]
    wpk = np.ascontiguousarray(wpk.reshape(2, 128, 256).transpose(1, 0, 2))
    w_v = np.asarray(w_v, dtype=np.float32)
    wv2 = np.concatenate([w_v, w_v])
    wvc = np.ascontiguousarray(wv2[:, None] * COEFFS[None, :] * QSCALE)
    in_maps = []
    for core in range(NCORES):
        b, hf = core // 2, core % 2
        L = int(valid_lens[b])
        mask = (np.arange(KBC) < L).astype(np.float32)[:, None]
        vaug = (np.concatenate([values[b, :KBC], np.ones((KBC, 1), np.float32)],
                               axis=1) * mask).astype(np.float16)
        qT = queries[b, hf * QH:(hf + 1) * QH, :].T.astype(np.float16)  # [256,QH]
        kT = keys[b, :KBC].T.astype(np.float16)                         # [256,KBC]
        in_maps.append({
            "queriesT": np.ascontiguousarray(
                qT.reshape(2, 128, QH).transpose(1, 0, 2)),
            "keysT": np.ascontiguousarray(
                kT.reshape(2, 128, KBC).transpose(1, 0, 2)),
            "vaug": np.ascontiguousarray(
                vaug.reshape(kb, 128, V + 1).transpose(1, 0, 2)),
            "wpk": wpk,
            "wvc": wvc,
        })
    return in_maps


def kernel(queries, keys, values, valid_lens, W_q, W_k, w_v):
    in_maps = make_in_maps(queries, keys, values, valid_lens, W_q, W_k, w_v)
    nc = get_nc()
    rkey = f"run_{_STATE['kb']}"
    if rkey not in _STATE:
        _STATE[rkey] = _build_runner(nc)
    results = _STATE[rkey](in_maps)
    out = np.empty((B, QFULL, V), np.float32)
    for core in range(NCORES):
        b, hf = core // 2, core % 2
        out[b, hf * QH:(hf + 1) * QH, :] = results[core]["out"].astype(np.float32)
    return out


# revision 60
# speedup vs baseline: 1.0349x; 1.0349x over previous
"""Additive-attention fused kernel for one TRN2 chip (8 NeuronCores).

Math (per batch b):
    q = queries @ W_q.T                       [Q, H]
    k = keys    @ W_k.T                       [K, H]
    scores[q,k] = sum_h w_v[h] * tanh(q[q,h] + k[k,h])
    attn = masked_softmax(scores, valid_len)  (mask: k >= L -> weight 0)
    out  = attn @ values                      [Q, V]

Sharding: data-parallel over batch B=4 x 2-way split of Q -> 8 cores,
each core handles [QH=512, :] of one batch.  No collectives needed.

v3 algorithm (v2 was a per-m DVE range-reduction + ACT Sin pipeline):

1. Sparse K: only kb = ceil(max(valid_lens)/128) k-blocks are live; the
   masked tail contributes exactly 0 through the pre-masked [values|1]
   operand, so keys/values beyond kb*128 are never shipped or touched.
2. tanh(z) ~ sum_m c_m sin(pi m z / FL) (odd Fourier series, weighted
   LSQ fit, M=8, |z| <= 2*BCLAMP), which makes scores ONE accumulated
   matmul over contraction (h, m, {sin,cos}).
3. Chebyshev recurrence for the features: with theta = 2 pi nu1 x,
       t_m = sin(m theta + phase),  t_{m+1} = 2 cos(theta) t_m - t_{m-1}
   so only m=1 needs ACT Sin (|arg| <= ~pi, where the table is exact);
   every further frequency is 2 cheap DVE ops (scalar_tensor_tensor +
   tensor_tensor) instead of range-reduction chains + Sin.
4. Hybrid fp8: frequencies m>=3 (small |c_m|) are quantized to fp8e4
   (q side pre-scaled x64) and their score contributions run as
   DoubleRow matmuls (2x PE throughput, contraction 256/instr); m=1,2
   stay fp16.  All q scales carry x64 so PSUM holds 64*scores; the exp
   reads PSUM with scale=1/64, bias=-2 (softmax shift for fp8 headroom
   elsewhere and smaller exp range).
5. Tail pipelining: score PSUM banks are split in two groups; exp of
   group A runs while PE still accumulates group B, then attn@V
   (fp16: fp8 weights/values fail the accuracy budget) accumulates
   per 128-query block with the denominator in an appended ones
   column; DVE reciprocal + gpsimd scale produce the fp16 output.

Engine budget per core (kb=6): DVE ~10us (recurrence), gpsimd ~9us
(scales/quantize/epilogue), PE ~9us (proj + scores + attn@V), ACT ~7us
(seeds + exp); HBM ~1.3MB.
"""

import math

import numpy as np

B, QFULL, KK = 4, 1024, 1024
D, H, V = 256, 64, 256
QH = 512            # Q rows per core
NCORES = 8

FM = 7              # number of frequencies
FL = 8.0            # half-period (= 2*BCLAMP so the m=1 seed args fit the
                    # Sin table domain [-pi, pi] exactly)
BCLAMP = 4.0        # clamp q/k projections to +-BCLAMP
MLOW = 5            # per-bank frequency stagger: the last two k-banks use
                    # MLOW frequencies so their exp hides under the recurrence
QSCALE = 64.0       # q-feature prescale (PSUM holds QSCALE*scores)
EXP_BIAS = -2.0     # softmax shift


def _bank_M(kb):
    if kb <= 4:
        return [FM] * kb
    return [FM if j < kb - 2 else MLOW for j in range(kb)]

_STATE = {}


def _fit_coeffs():
    z = np.linspace(-2 * BCLAMP, 2 * BCLAMP, 4001)
    w = np.exp(-z ** 2 / (2 * 2.2)) + 1e-4
    A = np.sin(np.pi / FL * np.outer(z, np.arange(1, FM + 1)))
    c = np.linalg.lstsq(A * np.sqrt(w)[:, None], np.tanh(z) * np.sqrt(w),
                        rcond=None)[0]
    return c.astype(np.float32)


COEFFS = _fit_coeffs()


def _fit_cospoly():
    """Even polynomial (deg 2 in x^2) for 2*cos(2 pi nu1 x), |x| <= BCLAMP."""
    x = np.linspace(-BCLAMP, BCLAMP, 2001)
    z = x * x
    A = np.stack([np.ones_like(z), z, z * z], axis=1)
    c = np.linalg.lstsq(A, 2.0 * np.cos(np.pi / FL * x), rcond=None)[0]
    return c.astype(np.float64)


COSP = _fit_cospoly()


def _build_nc(kb, n_iters=1):
    import contextlib
    import concourse.tile as tile
    from concourse import bacc, mybir

    F32 = mybir.dt.float32
    F16 = mybir.dt.float16
    F8 = mybir.dt.float8e4
    Sin = mybir.ActivationFunctionType.Sin
    Exp = mybir.ActivationFunctionType.Exp
    AOp = mybir.AluOpType
    DR = mybir.MatmulPerfMode.DoubleRow
    TWO_PI = 2.0 * math.pi

    KBC = kb * 128                  # live key columns
    W = QH + KBC                    # feature width (q cols | k cols)
    NU1 = 1.0 / (2.0 * FL)
    BANK_M = _bank_M(kb)
    if kb <= 4:
        GROUPS = [(j0, min(j0 + 2, kb)) for j0 in range(0, kb, 2)]
    else:
        GROUPS = [(0, 2), (2, 4), (4, kb)]
    NGRP = len(GROUPS)

    nc = bacc.Bacc()
    # host layouts are partition-major so each tensor is ONE DMA:
    # qT [128, 2, QH]: [p, c, :] = queries.T fp16 rows c*128+p
    # wpk [128, 2, 256]: [p, c, 0:128] = wqT2 rows, [.., 128:256] = wkT2
    #   (w*T2 = [D, 128] with the 64 output rows duplicated -> projections
    #    emit all 128 feature partitions directly, no SBUF dup pass)
    qT_d = nc.declare_dram_parameter("queriesT", [128, 2, QH], F16, isOutput=False)
    kT_d = nc.declare_dram_parameter("keysT", [128, 2, KBC], F16, isOutput=False)
    va_d = nc.declare_dram_parameter("vaug", [128, kb, V + 1], F16, isOutput=False)
    wpk_d = nc.declare_dram_parameter("wpk", [128, 2, 256], F16, isOutput=False)
    wvc_d = nc.declare_dram_parameter("wvc", [128, FM], F32, isOutput=False)
    out_d = nc.declare_dram_parameter("out", [QH, V], F16, isOutput=True)

    with tile.TileContext(nc) as tc:
        with (
            tc.tile_pool(name="singles", bufs=1) as singles,
            tc.tile_pool(name="tpool", bufs=5) as tpool,
            tc.tile_pool(name="upool", bufs=2) as upool,
            tc.tile_pool(name="qk8", bufs=1) as qk8,
            tc.tile_pool(name="outp", bufs=2) as outp,
            # one PSUM pool; per-tag slots: score group g owns tag "scg{g}",
            # projections and attn@V accumulators reuse those banks via the
            # same tags (Tile serializes on the tag's reader/writer chain)
            tc.tile_pool(name="ps_big", bufs=1, space="PSUM") as ps_big,
            tc.For_i(0, n_iters, 1,
                     hint_engines=(mybir.EngineType.PE, mybir.EngineType.DVE,
                                   mybir.EngineType.Activation,
                                   mybir.EngineType.SP, mybir.EngineType.Pool),
                     staggered_reset=True)
            if n_iters > 1 else contextlib.nullcontext(),
        ):
            # -------- stage inputs (5 DMAs total, split across 2 queues) ----
            wpk = singles.tile([128, 2, 256], F16, tag="wpk", name="wpk")
            nc.sync.dma_start(wpk, wpk_d[:, :, :])
            qTt = singles.tile([128, 2, QH], F16, tag="qTt", name="qTt")
            nc.sync.dma_start(qTt, qT_d[:, :, :])
            kTt = singles.tile([128, 2, KBC], F16, tag="kTt", name="kTt")
            nc.sync.dma_start(kTt, kT_d[:, :, :])
            wvc_sb = singles.tile([128, FM], F32, tag="wvc", name="wvc")
            nc.sync.dma_start(wvc_sb, wvc_d[:, :])
            v_aug = singles.tile([128, kb, V + 1], F16, tag="vaug", name="vaug")
            nc.sync.dma_start(v_aug, va_d[:, :, :])

            # phase biases (radians) for the m=1 seeds
            bq = singles.tile([128, 1], F32, tag="bq", name="bq")
            nc.vector.memset(bq[0:H, :], 0.0)
            nc.vector.memset(bq[H:128, :], math.pi / 2)
            bk = singles.tile([128, 1], F32, tag="bk", name="bk")
            nc.vector.memset(bk[0:H, :], math.pi / 2)
            nc.vector.memset(bk[H:128, :], 0.0)
            bc = singles.tile([128, 1], F32, tag="bc", name="bc")
            nc.vector.memset(bc, math.pi / 2)
            be = singles.tile([128, 1], F32, tag="be", name="be")
            nc.vector.memset(be, EXP_BIAS)
            # t_0 per-partition constants (m=2 seed): q side [0;1], k side [1;0]
            t0q = singles.tile([128, 1], F32, tag="t0q", name="t0q")
            nc.vector.memset(t0q[0:H, :], 0.0)
            nc.vector.memset(t0q[H:128, :], 1.0)
            t0k = singles.tile([128, 1], F32, tag="t0k", name="t0k")
            nc.vector.memset(t0k[0:H, :], 1.0)
            nc.vector.memset(t0k[H:128, :], 0.0)
            # dummy Sin to prefetch the trig act table before xt is ready
            dum = singles.tile([128, 1], F32, tag="dum", name="dum")
            nc.scalar.activation(dum, bq[:, 0:1], Sin)

            # ------------- projections + clamp -> fp16 [128, cols] -------------
            # W rows are host-duplicated so the matmul emits all 128 feature
            # partitions; separate per-side tiles avoid false tile deps.
            # each matmul output must stay inside one 512-float PSUM bank.
            xtq = singles.tile([128, QH], F16, tag="xtq", name="xtq")
            xtk = singles.tile([128, KBC], F16, tag="xtk", name="xtk")
            nkc = -(-KBC // 512)
            psq = ps_big.tile([128, 512], F32, tag=f"scg{NGRP - 1}", name="psq")
            for c in range(2):
                nc.tensor.matmul(psq, wpk[:, c, 0:128], qTt[:, c, :],
                                 start=(c == 0), stop=(c == 1))
            nc.vector.tensor_scalar(xtq, psq, BCLAMP, -BCLAMP, AOp.min, AOp.max)
            psk = ps_big.tile([128, nkc * 512], F32, tag="scg0", name="psk")
            for ci in range(nkc):
                c0, cw = ci * 512, min(512, KBC - ci * 512)
                for c in range(2):
                    nc.tensor.matmul(psk[:, c0:c0 + cw], wpk[:, c, 128:256],
                                     kTt[:, c, c0:c0 + cw],
                                     start=(c == 0), stop=(c == 1))
            # (gpsimd cannot touch PSUM on hw; clamp-k goes to DVE)
            nc.vector.tensor_scalar(xtk, psk[:, 0:KBC], BCLAMP,
                                    -BCLAMP, AOp.min, AOp.max)

            # ------------- m=1 seeds + 2cos multiplier -------------
            # t1 = sin(2 pi nu1 x + phase); the cos rows already live inside
            # t1 (q side rows 64:128, k side rows 0:64), so c2d = 2cos comes
            # from partition-shift SBUF DMAs + one DVE scale, not two more
            # ACT Sin calls.
            t1 = singles.tile([128, W], F16, tag="t1", name="t1")
            nc.scalar.activation(t1[:, 0:QH], xtq, Sin,
                                 bias=bq[:, 0:1], scale=TWO_PI * NU1)
            nc.scalar.activation(t1[:, QH:W], xtk, Sin,
                                 bias=bk[:, 0:1], scale=TWO_PI * NU1)
            c1dup = singles.tile([128, W], F16, tag="c1dup", name="c1dup")
            nc.sync.dma_start(c1dup[0:H, 0:QH], t1[H:128, 0:QH])
            nc.sync.dma_start(c1dup[H:128, 0:QH], t1[H:128, 0:QH])
            nc.sync.dma_start(c1dup[0:H, QH:W], t1[0:H, QH:W])
            nc.sync.dma_start(c1dup[H:128, QH:W], t1[0:H, QH:W])
            c2d = singles.tile([128, W], F16, tag="c2d", name="c2d")
            nc.vector.tensor_scalar(c2d, c1dup, 2.0, None, AOp.mult)

            # ------------- features: Chebyshev recurrence + scales -------------
            # per m: u = t_{m-1} . c2d ; t_m = u - t_{m-2}   (DVE, all fp16)
            # plus qf_m = t_m[:, :QH] * (w_v c_m 64) for the score matmuls.
            # 3-level per-bank frequency stagger: each group's exp fires as
            # soon as its banks close, hiding under the rest of the recurrence
            bank_M = BANK_M
            qf16 = {}
            tms = {1: t1}
            for m in range(1, FM + 1):
                if m >= 2:
                    u = upool.tile([128, W], F16, tag="u", name=f"u{m}")
                    nc.vector.tensor_tensor(u, tms[m - 1], c2d, AOp.mult)
                    # every t_m is a PE matmul operand later; keep them all
                    tm = singles.tile([128, W], F16, tag=f"t{m}", name=f"t{m}")
                    if m == 2:
                        # t2 = u - t0 with t0 constant per partition-half
                        nc.vector.tensor_scalar(tm[:, 0:QH], u[:, 0:QH],
                                                t0q[:, 0:1], None, AOp.subtract)
                        nc.vector.tensor_scalar(tm[:, QH:W], u[:, QH:W],
                                                t0k[:, 0:1], None, AOp.subtract)
                    else:
                        nc.vector.tensor_tensor(tm, u, tms[m - 2], AOp.subtract)
                    tms[m] = tm
                tm = tms[m]
                qf = singles.tile([128, QH], F16, tag=f"qf{m}", name=f"qf{m}")
                nc.vector.tensor_scalar(qf, tm[:, 0:QH],
                                        wvc_sb[:, m - 1:m], None, AOp.mult)
                qf16[m] = qf

            # ------------- scores (PSUM accumulate), staggered tail -------------
            # banks grouped into per-tag PSUM tiles: full-M banks in pairs,
            # the LOWB low-M banks as the last group.  m-major emission =
            # matmuls fire the moment qf_m lands; low banks close at MLOW and
            # their exp hides under the rest of the recurrence.
            groups = GROUPS
            scg = []
            for g, (j0, j1) in enumerate(groups):
                scg.append(ps_big.tile([128, j1 - j0, QH], F32, tag=f"scg{g}",
                                       name=f"scg{g}"))
            jmap = {}
            group_M = []
            for g, (j0, j1) in enumerate(groups):
                for j in range(j0, j1):
                    jmap[j] = (g, j - j0)
                group_M.append(max(bank_M[j] for j in range(j0, j1)))
            p16 = [None] * NGRP
            for m in range(1, FM + 1):
                for j in range(kb):
                    if m > bank_M[j]:
                        continue
                    g, ji = jmap[j]
                    nc.tensor.matmul(
                        scg[g][:, ji, :],
                        tms[m][:, QH + j * 128:QH + (j + 1) * 128],
                        qf16[m], start=(m == 1), stop=(m == bank_M[j]))
                # emit each group's exp right after its last m-sweep so it
                # runs during the remaining recurrence
                for g, (j0, j1) in enumerate(groups):
                    if group_M[g] != m:
                        continue
                    pg = singles.tile([128, j1 - j0, QH], F16, tag=f"p16_{g}",
                                      name=f"p16_{g}")
                    nc.scalar.activation(pg.rearrange("p a b -> p (a b)"),
                                         scg[g].rearrange("p a b -> p (a b)"),
                                         Exp, bias=be[:, 0:1],
                                         scale=1.0 / QSCALE)
                    p16[g] = pg

            # ------------- attn @ [values | ones] + epilogue -------------
            # 4 concurrent accumulators: first pair in the last group's
            # banks (freed by the earliest exp), second in the mid group's;
            # j-major emission, latest-closing banks last
            av01 = ps_big.tile([128, 2, 512], F32, tag=f"scg{NGRP - 1}",
                               name="av01")
            av23 = ps_big.tile([128, 2, 512], F32,
                               tag=f"scg{1 if NGRP >= 2 else 0}", name="av23")
            avs = [av01[:, 0, 0:V + 1], av01[:, 1, 0:V + 1],
                   av23[:, 0, 0:V + 1], av23[:, 1, 0:V + 1]]
            jorder = sorted(range(kb), key=lambda j: (bank_M[j], -j))
            for idx, j in enumerate(jorder):
                g, ji = jmap[j]
                pj = p16[g][:, ji, :]
                for qb in range(QH // 128):
                    nc.tensor.matmul(avs[qb], pj[:, qb * 128:(qb + 1) * 128],
                                     v_aug[:, j, :], start=(idx == 0),
                                     stop=(idx == kb - 1))
            # output in two half-tiles so the first DMA overlaps the rest;
            # epilogue on DVE (reciprocal + scale), ACT is done after exps
            o16h = [outp.tile([128, 2, V], F16, tag=f"o16_{h}", name=f"o16_{h}")
                    for h in range(2)]
            for qb in range(QH // 128):
                av = avs[qb]
                rcp = outp.tile([128, 1], F32, tag="rcp", name=f"rcp{qb}")
                nc.vector.reciprocal(rcp, av[:, V:V + 1])
                nc.vector.tensor_scalar(o16h[qb // 2][:, qb % 2, :], av[:, 0:V],
                                        rcp[:, 0:1], None, AOp.mult)
                if qb % 2 == 1:
                    nc.sync.dma_start(
                        out_d.rearrange("(a p) v -> p a v", p=128)
                        [:, (qb - 1):(qb + 1), :], o16h[qb // 2])

    nc.finalize()
    return nc


def _build_runner(nc):
    """Cached multi-core PJRT runner (keeps the jitted callable so repeat
    calls don't retrace/recompile)."""
    import jax
    import numpy as _np
    from jax.sharding import Mesh, PartitionSpec
    from jax.experimental.shard_map import shard_map
    from concourse import bass2jax, mybir

    bass2jax.install_neuronx_cc_hook()

    partition_name = nc.partition_id_tensor.name if nc.partition_id_tensor else None
    in_names, out_names, out_avals, zero_outs = [], [], [], []
    for alloc in nc.m.functions[0].allocations:
        if not isinstance(alloc, mybir.MemoryLocationSet):
            continue
        name = alloc.memorylocations[0].name
        if alloc.kind == "ExternalInput":
            if name != partition_name:
                in_names.append(name)
        elif alloc.kind == "ExternalOutput":
            shape = tuple(alloc.tensor_shape)
            dtype = mybir.dt.np(alloc.dtype)
            out_names.append(name)
            out_avals.append(jax.core.ShapedArray(shape, dtype))
            zero_outs.append(_np.zeros(shape, dtype))
    n_params = len(in_names)
    n_outs = len(out_avals)
    all_in_names = list(in_names) + list(out_names)
    if partition_name is not None:
        all_in_names.append(partition_name)
    donate = tuple(range(n_params, n_params + n_outs))

    def _body(*args):
        operands = list(args)
        if partition_name is not None:
            operands.append(bass2jax.partition_id_tensor())
        outs = bass2jax._bass_exec_p.bind(
            *operands,
            out_avals=tuple(out_avals),
            in_names=tuple(all_in_names),
            out_names=tuple(out_names),
            lowering_input_output_aliases=(),
            sim_require_finite=True,
            sim_require_nnan=True,
            nc=nc,
        )
        return tuple(outs)

    devices = jax.devices()[:NCORES]
    assert len(devices) == NCORES, f"need {NCORES} cores, have {len(jax.devices())}"
    mesh = Mesh(_np.asarray(devices), ("core",))
    in_specs = (PartitionSpec("core"),) * (n_params + n_outs)
    out_specs = (PartitionSpec("core"),) * n_outs
    sharded = jax.jit(
        shard_map(_body, mesh=mesh, in_specs=in_specs, out_specs=out_specs,
                  check_rep=False),
        donate_argnums=donate, keep_unused=True)

    def run(in_maps):
        per_core = [[_np.asarray(m[name]) for name in in_names] for m in in_maps]
        concat_in = [
            _np.concatenate([per_core[c][i] for c in range(NCORES)], axis=0)
            for i in range(n_params)
        ]
        concat_zeros = [
            _np.zeros((NCORES * z.shape[0], *z.shape[1:]), z.dtype) for z in zero_outs
        ]
        out_arrs = sharded(*concat_in, *concat_zeros)
        return [
            {
                name: _np.asarray(out_arrs[i]).reshape(NCORES, *out_avals[i].shape)[c]
                for i, name in enumerate(out_names)
            }
            for c in range(NCORES)
        ]

    return run


def get_nc(n_iters=1, kb=None):
    if kb is None:
        kb = _STATE.get("kb", 6)
    key = f"nc{n_iters}_{kb}"
    if key not in _STATE:
        _STATE[key] = _build_nc(kb, n_iters)
    return _STATE[key]


def make_in_maps(queries, keys, values, valid_lens, W_q, W_k, w_v):
    queries = np.asarray(queries, dtype=np.float32)
    keys = np.asarray(keys, dtype=np.float32)
    values = np.asarray(values, dtype=np.float32)
    valid_lens = np.asarray(valid_lens)
    kb = max(1, min(KK // 128, int(-(-int(valid_lens.max()) // 128))))
    _STATE["kb"] = kb
    KBC = kb * 128
    # weights pack: W.T with output rows duplicated (128 feature rows),
    # chunked partition-major: wpk[p, c, 0:128] = wqT2 row c*128+p
    WqT2 = np.concatenate([np.asarray(W_q, np.float32).T] * 2, axis=1)  # [256,128]
    WkT2 = np.concatenate([np.asarray(W_k, np.float32).T] * 2, axis=1)
    wpk = np.concatenate([WqT2, WkT2], axis=1).astype(np.float16)      # [256,256]
    wpk = np.ascontiguousarray(wpk.reshape(2, 128, 256).transpose(1, 0, 2))
    w_v = np.asarray(w_v, dtype=np.float32)
    wv2 = np.concatenate([w_v, w_v])
    wvc = np.ascontiguousarray(wv2[:, None] * COEFFS[None, :] * QSCALE)
    in_maps = []
    for core in range(NCORES):
        b, hf = core // 2, core % 2
        L = int(valid_lens[b])
        mask = (np.arange(KBC) < L).astype(np.float32)[:, None]
        vaug = (np.concatenate([values[b, :KBC], np.ones((KBC, 1), np.float32)],
                               axis=1) * mask).astype(np.float16)
        qT = queries[b, hf * QH:(hf + 1) * QH, :].T.astype(np.float16)  # [256,QH]
        kT = keys[b, :KBC].T.astype(np.float16)                         # [256,KBC]
        in_maps.append({
            "queriesT": np.ascontiguousarray(
                qT.reshape(2, 128, QH).transpose(1, 0, 2)),
            "keysT": np.ascontiguousarray(
                kT.reshape(2, 128, KBC).transpose(1, 0, 2)),
            "vaug": np.ascontiguousarray(
                vaug.reshape(kb, 128, V + 1).transpose(1, 0, 2)),
            "wpk": wpk,
            "wvc": wvc,
        })
    return in_maps


def kernel(queries, keys, values, valid_lens, W_q, W_k, w_v):
    in_maps = make_in_maps(queries, keys, values, valid_lens, W_q, W_k, w_v)
    nc = get_nc()
    rkey = f"run_{_STATE['kb']}"
    if rkey not in _STATE:
        _STATE[rkey] = _build_runner(nc)
    results = _STATE[rkey](in_maps)
    out = np.empty((B, QFULL, V), np.float32)
    for core in range(NCORES):
        b, hf = core // 2, core % 2
        out[b, hf * QH:(hf + 1) * QH, :] = results[core]["out"].astype(np.float32)
    return out


# revision 61
# speedup vs baseline: 1.0670x; 1.0310x over previous
"""Additive-attention fused kernel for one TRN2 chip (8 NeuronCores).

Math (per batch b):
    q = queries @ W_q.T                       [Q, H]
    k = keys    @ W_k.T                       [K, H]
    scores[q,k] = sum_h w_v[h] * tanh(q[q,h] + k[k,h])
    attn = masked_softmax(scores, valid_len)  (mask: k >= L -> weight 0)
    out  = attn @ values                      [Q, V]

Sharding: data-parallel over batch B=4 x 2-way split of Q -> 8 cores,
each core handles [QH=512, :] of one batch.  No collectives needed.

v3 algorithm (v2 was a per-m DVE range-reduction + ACT Sin pipeline):

1. Sparse K: only kb = ceil(max(valid_lens)/128) k-blocks are live; the
   masked tail contributes exactly 0 through the pre-masked [values|1]
   operand, so keys/values beyond kb*128 are never shipped or touched.
2. tanh(z) ~ sum_m c_m sin(pi m z / FL) (odd Fourier series, weighted
   LSQ fit, M=8, |z| <= 2*BCLAMP), which makes scores ONE accumulated
   matmul over contraction (h, m, {sin,cos}).
3. Chebyshev recurrence for the features: with theta = 2 pi nu1 x,
       t_m = sin(m theta + phase),  t_{m+1} = 2 cos(theta) t_m - t_{m-1}
   so only m=1 needs ACT Sin (|arg| <= ~pi, where the table is exact);
   every further frequency is 2 cheap DVE ops (scalar_tensor_tensor +
   tensor_tensor) instead of range-reduction chains + Sin.
4. Hybrid fp8: frequencies m>=3 (small |c_m|) are quantized to fp8e4
   (q side pre-scaled x64) and their score contributions run as
   DoubleRow matmuls (2x PE throughput, contraction 256/instr); m=1,2
   stay fp16.  All q scales carry x64 so PSUM holds 64*scores; the exp
   reads PSUM with scale=1/64, bias=-2 (softmax shift for fp8 headroom
   elsewhere and smaller exp range).
5. Tail pipelining: score PSUM banks are split in two groups; exp of
   group A runs while PE still accumulates group B, then attn@V
   (fp16: fp8 weights/values fail the accuracy budget) accumulates
   per 128-query block with the denominator in an appended ones
   column; DVE reciprocal + gpsimd scale produce the fp16 output.

Engine budget per core (kb=6): DVE ~10us (recurrence), gpsimd ~9us
(scales/quantize/epilogue), PE ~9us (proj + scores + attn@V), ACT ~7us
(seeds + exp); HBM ~1.3MB.
"""

import math

import numpy as np

B, QFULL, KK = 4, 1024, 1024
D, H, V = 256, 64, 256
QH = 512            # Q rows per core
NCORES = 8

FM = 6              # number of frequencies
FL = 7.2            # half-period (= 2*BCLAMP)
                    # Sin table domain [-pi, pi] exactly)
BCLAMP = 3.6        # clamp q/k projections to +-BCLAMP
MLOW = 5            # per-bank frequency stagger: the last two k-banks use
                    # MLOW frequencies so their exp hides under the recurrence
QSCALE = 64.0       # q-feature prescale (PSUM holds QSCALE*scores)
EXP_BIAS = -2.0     # softmax shift


def _bank_M(kb):
    if kb <= 4:
        return [FM] * kb
    return [FM if j < kb - 2 else MLOW for j in range(kb)]

_STATE = {}


def _fit_coeffs():
    z = np.linspace(-2 * BCLAMP, 2 * BCLAMP, 4001)
    w = np.exp(-z ** 2 / (2 * 2.2)) + 1e-4
    A = np.sin(np.pi / FL * np.outer(z, np.arange(1, FM + 1)))
    c = np.linalg.lstsq(A * np.sqrt(w)[:, None], np.tanh(z) * np.sqrt(w),
                        rcond=None)[0]
    return c.astype(np.float32)


COEFFS = _fit_coeffs()


def _fit_cospoly():
    """Even polynomial (deg 2 in x^2) for 2*cos(2 pi nu1 x), |x| <= BCLAMP."""
    x = np.linspace(-BCLAMP, BCLAMP, 2001)
    z = x * x
    A = np.stack([np.ones_like(z), z, z * z], axis=1)
    c = np.linalg.lstsq(A, 2.0 * np.cos(np.pi / FL * x), rcond=None)[0]
    return c.astype(np.float64)


COSP = _fit_cospoly()


def _build_nc(kb, n_iters=1):
    import contextlib
    import concourse.tile as tile
    from concourse import bacc, mybir

    F32 = mybir.dt.float32
    F16 = mybir.dt.float16
    F8 = mybir.dt.float8e4
    Sin = mybir.ActivationFunctionType.Sin
    Exp = mybir.ActivationFunctionType.Exp
    AOp = mybir.AluOpType
    DR = mybir.MatmulPerfMode.DoubleRow
    TWO_PI = 2.0 * math.pi

    KBC = kb * 128                  # live key columns
    W = QH + KBC                    # feature width (q cols | k cols)
    NU1 = 1.0 / (2.0 * FL)
    BANK_M = _bank_M(kb)
    if kb <= 4:
        GROUPS = [(j0, min(j0 + 2, kb)) for j0 in range(0, kb, 2)]
    else:
        GROUPS = [(0, 2), (2, 4), (4, kb)]
    NGRP = len(GROUPS)

    nc = bacc.Bacc()
    # host layouts are partition-major so each tensor is ONE DMA:
    # qT [128, 2, QH]: [p, c, :] = queries.T fp16 rows c*128+p
    # wpk [128, 2, 256]: [p, c, 0:128] = wqT2 rows, [.., 128:256] = wkT2
    #   (w*T2 = [D, 128] with the 64 output rows duplicated -> projections
    #    emit all 128 feature partitions directly, no SBUF dup pass)
    qT_d = nc.declare_dram_parameter("queriesT", [128, 2, QH], F16, isOutput=False)
    kT_d = nc.declare_dram_parameter("keysT", [128, 2, KBC], F16, isOutput=False)
    va_d = nc.declare_dram_parameter("vaug", [128, kb, V + 1], F16, isOutput=False)
    wpk_d = nc.declare_dram_parameter("wpk", [128, 2, 256], F16, isOutput=False)
    wvc_d = nc.declare_dram_parameter("wvc", [128, FM], F32, isOutput=False)
    out_d = nc.declare_dram_parameter("out", [QH, V], F16, isOutput=True)

    with tile.TileContext(nc) as tc:
        with (
            tc.tile_pool(name="singles", bufs=1) as singles,
            tc.tile_pool(name="tpool", bufs=5) as tpool,
            tc.tile_pool(name="upool", bufs=2) as upool,
            tc.tile_pool(name="qk8", bufs=1) as qk8,
            tc.tile_pool(name="outp", bufs=2) as outp,
            # one PSUM pool; per-tag slots: score group g owns tag "scg{g}",
            # projections and attn@V accumulators reuse those banks via the
            # same tags (Tile serializes on the tag's reader/writer chain)
            tc.tile_pool(name="ps_big", bufs=1, space="PSUM") as ps_big,
            tc.For_i(0, n_iters, 1,
                     hint_engines=(mybir.EngineType.PE, mybir.EngineType.DVE,
                                   mybir.EngineType.Activation,
                                   mybir.EngineType.SP, mybir.EngineType.Pool),
                     staggered_reset=True)
            if n_iters > 1 else contextlib.nullcontext(),
        ):
            # -------- stage inputs (5 DMAs total, split across 2 queues) ----
            wpk = singles.tile([128, 2, 256], F16, tag="wpk", name="wpk")
            nc.sync.dma_start(wpk, wpk_d[:, :, :])
            qTt = singles.tile([128, 2, QH], F16, tag="qTt", name="qTt")
            nc.sync.dma_start(qTt, qT_d[:, :, :])
            kTt = singles.tile([128, 2, KBC], F16, tag="kTt", name="kTt")
            nc.sync.dma_start(kTt, kT_d[:, :, :])
            wvc_sb = singles.tile([128, FM], F32, tag="wvc", name="wvc")
            nc.sync.dma_start(wvc_sb, wvc_d[:, :])
            v_aug = singles.tile([128, kb, V + 1], F16, tag="vaug", name="vaug")
            nc.sync.dma_start(v_aug, va_d[:, :, :])

            # phase biases (radians) for the m=1 seeds
            bq = singles.tile([128, 1], F32, tag="bq", name="bq")
            nc.vector.memset(bq[0:H, :], 0.0)
            nc.vector.memset(bq[H:128, :], math.pi / 2)
            bk = singles.tile([128, 1], F32, tag="bk", name="bk")
            nc.vector.memset(bk[0:H, :], math.pi / 2)
            nc.vector.memset(bk[H:128, :], 0.0)
            bc = singles.tile([128, 1], F32, tag="bc", name="bc")
            nc.vector.memset(bc, math.pi / 2)
            be = singles.tile([128, 1], F32, tag="be", name="be")
            nc.vector.memset(be, EXP_BIAS)
            # t_0 per-partition constants (m=2 seed): q side [0;1], k side [1;0]
            t0q = singles.tile([128, 1], F32, tag="t0q", name="t0q")
            nc.vector.memset(t0q[0:H, :], 0.0)
            nc.vector.memset(t0q[H:128, :], 1.0)
            t0k = singles.tile([128, 1], F32, tag="t0k", name="t0k")
            nc.vector.memset(t0k[0:H, :], 1.0)
            nc.vector.memset(t0k[H:128, :], 0.0)
            # dummy Sin to prefetch the trig act table before xt is ready
            dum = singles.tile([128, 1], F32, tag="dum", name="dum")
            nc.scalar.activation(dum, bq[:, 0:1], Sin)

            # ------------- projections + clamp -> fp16 [128, cols] -------------
            # W rows are host-duplicated so the matmul emits all 128 feature
            # partitions; separate per-side tiles avoid false tile deps.
            # each matmul output must stay inside one 512-float PSUM bank.
            xtq = singles.tile([128, QH], F16, tag="xtq", name="xtq")
            xtk = singles.tile([128, KBC], F16, tag="xtk", name="xtk")
            nkc = -(-KBC // 512)
            psq = ps_big.tile([128, 512], F32, tag=f"scg{NGRP - 1}", name="psq")
            for c in range(2):
                nc.tensor.matmul(psq, wpk[:, c, 0:128], qTt[:, c, :],
                                 start=(c == 0), stop=(c == 1))
            nc.vector.tensor_scalar(xtq, psq, BCLAMP, -BCLAMP, AOp.min, AOp.max)
            psk = ps_big.tile([128, nkc * 512], F32, tag="scg0", name="psk")
            for ci in range(nkc):
                c0, cw = ci * 512, min(512, KBC - ci * 512)
                for c in range(2):
                    nc.tensor.matmul(psk[:, c0:c0 + cw], wpk[:, c, 128:256],
                                     kTt[:, c, c0:c0 + cw],
                                     start=(c == 0), stop=(c == 1))
            # (gpsimd cannot touch PSUM on hw; clamp-k goes to DVE)
            nc.vector.tensor_scalar(xtk, psk[:, 0:KBC], BCLAMP,
                                    -BCLAMP, AOp.min, AOp.max)

            # ------------- m=1 seeds + 2cos multiplier -------------
            # t1 = sin(2 pi nu1 x + phase); the cos rows already live inside
            # t1 (q side rows 64:128, k side rows 0:64), so c2d = 2cos comes
            # from partition-shift SBUF DMAs + one DVE scale, not two more
            # ACT Sin calls.
            t1 = singles.tile([128, W], F16, tag="t1", name="t1")
            nc.scalar.activation(t1[:, 0:QH], xtq, Sin,
                                 bias=bq[:, 0:1], scale=TWO_PI * NU1)
            nc.scalar.activation(t1[:, QH:W], xtk, Sin,
                                 bias=bk[:, 0:1], scale=TWO_PI * NU1)
            c1dup = singles.tile([128, W], F16, tag="c1dup", name="c1dup")
            nc.sync.dma_start(c1dup[0:H, 0:QH], t1[H:128, 0:QH])
            nc.sync.dma_start(c1dup[H:128, 0:QH], t1[H:128, 0:QH])
            nc.sync.dma_start(c1dup[0:H, QH:W], t1[0:H, QH:W])
            nc.sync.dma_start(c1dup[H:128, QH:W], t1[0:H, QH:W])
            c2d = singles.tile([128, W], F16, tag="c2d", name="c2d")
            nc.vector.tensor_scalar(c2d, c1dup, 2.0, None, AOp.mult)

            # ------------- features: Chebyshev recurrence + scales -------------
            # per m: u = t_{m-1} . c2d ; t_m = u - t_{m-2}   (DVE, all fp16)
            # plus qf_m = t_m[:, :QH] * (w_v c_m 64) for the score matmuls.
            # 3-level per-bank frequency stagger: each group's exp fires as
            # soon as its banks close, hiding under the rest of the recurrence
            bank_M = BANK_M
            qf16 = {}
            tms = {1: t1}
            for m in range(1, FM + 1):
                if m >= 2:
                    u = upool.tile([128, W], F16, tag="u", name=f"u{m}")
                    nc.vector.tensor_tensor(u, tms[m - 1], c2d, AOp.mult)
                    # every t_m is a PE matmul operand later; keep them all
                    tm = singles.tile([128, W], F16, tag=f"t{m}", name=f"t{m}")
                    if m == 2:
                        # t2 = u - t0 with t0 constant per partition-half
                        nc.vector.tensor_scalar(tm[:, 0:QH], u[:, 0:QH],
                                                t0q[:, 0:1], None, AOp.subtract)
                        nc.vector.tensor_scalar(tm[:, QH:W], u[:, QH:W],
                                                t0k[:, 0:1], None, AOp.subtract)
                    else:
                        nc.vector.tensor_tensor(tm, u, tms[m - 2], AOp.subtract)
                    tms[m] = tm
                tm = tms[m]
                qf = singles.tile([128, QH], F16, tag=f"qf{m}", name=f"qf{m}")
                nc.vector.tensor_scalar(qf, tm[:, 0:QH],
                                        wvc_sb[:, m - 1:m], None, AOp.mult)
                qf16[m] = qf

            # ------------- scores (PSUM accumulate), staggered tail -------------
            # banks grouped into per-tag PSUM tiles: full-M banks in pairs,
            # the LOWB low-M banks as the last group.  m-major emission =
            # matmuls fire the moment qf_m lands; low banks close at MLOW and
            # their exp hides under the rest of the recurrence.
            groups = GROUPS
            scg = []
            for g, (j0, j1) in enumerate(groups):
                scg.append(ps_big.tile([128, j1 - j0, QH], F32, tag=f"scg{g}",
                                       name=f"scg{g}"))
            jmap = {}
            group_M = []
            for g, (j0, j1) in enumerate(groups):
                for j in range(j0, j1):
                    jmap[j] = (g, j - j0)
                group_M.append(max(bank_M[j] for j in range(j0, j1)))
            p16 = [None] * NGRP
            for m in range(1, FM + 1):
                for j in range(kb):
                    if m > bank_M[j]:
                        continue
                    g, ji = jmap[j]
                    nc.tensor.matmul(
                        scg[g][:, ji, :],
                        tms[m][:, QH + j * 128:QH + (j + 1) * 128],
                        qf16[m], start=(m == 1), stop=(m == bank_M[j]))
                # emit each group's exp right after its last m-sweep so it
                # runs during the remaining recurrence
                for g, (j0, j1) in enumerate(groups):
                    if group_M[g] != m:
                        continue
                    pg = singles.tile([128, j1 - j0, QH], F16, tag=f"p16_{g}",
                                      name=f"p16_{g}")
                    nc.scalar.activation(pg.rearrange("p a b -> p (a b)"),
                                         scg[g].rearrange("p a b -> p (a b)"),
                                         Exp, bias=be[:, 0:1],
                                         scale=1.0 / QSCALE)
                    p16[g] = pg

            # ------------- attn @ [values | ones] + epilogue -------------
            # 4 concurrent accumulators: first pair in the last group's
            # banks (freed by the earliest exp), second in the mid group's;
            # j-major emission, latest-closing banks last
            av01 = ps_big.tile([128, 2, 512], F32, tag=f"scg{NGRP - 1}",
                               name="av01")
            av23 = ps_big.tile([128, 2, 512], F32,
                               tag=f"scg{1 if NGRP >= 2 else 0}", name="av23")
            avs = [av01[:, 0, 0:V + 1], av01[:, 1, 0:V + 1],
                   av23[:, 0, 0:V + 1], av23[:, 1, 0:V + 1]]
            jorder = sorted(range(kb), key=lambda j: (bank_M[j], -j))
            for idx, j in enumerate(jorder):
                g, ji = jmap[j]
                pj = p16[g][:, ji, :]
                for qb in range(QH // 128):
                    nc.tensor.matmul(avs[qb], pj[:, qb * 128:(qb + 1) * 128],
                                     v_aug[:, j, :], start=(idx == 0),
                                     stop=(idx == kb - 1))
            # output in two half-tiles so the first DMA overlaps the rest;
            # epilogue on DVE (reciprocal + scale), ACT is done after exps
            o16h = [outp.tile([128, 2, V], F16, tag=f"o16_{h}", name=f"o16_{h}")
                    for h in range(2)]
            for qb in range(QH // 128):
                av = avs[qb]
                rcp = outp.tile([128, 1], F32, tag="rcp", name=f"rcp{qb}")
                nc.vector.reciprocal(rcp, av[:, V:V + 1])
                nc.vector.tensor_scalar(o16h[qb // 2][:, qb % 2, :], av[:, 0:V],
                                        rcp[:, 0:1], None, AOp.mult)
                if qb % 2 == 1:
                    nc.sync.dma_start(
                        out_d.rearrange("(a p) v -> p a v", p=128)
                        [:, (qb - 1):(qb + 1), :], o16h[qb // 2])

    nc.finalize()
    return nc


def _build_runner(nc):
    """Cached multi-core PJRT runner (keeps the jitted callable so repeat
    calls don't retrace/recompile)."""
    import jax
    import numpy as _np
    from jax.sharding import Mesh, PartitionSpec
    from jax.experimental.shard_map import shard_map
    from concourse import bass2jax, mybir

    bass2jax.install_neuronx_cc_hook()

    partition_name = nc.partition_id_tensor.name if nc.partition_id_tensor else None
    in_names, out_names, out_avals, zero_outs = [], [], [], []
    for alloc in nc.m.functions[0].allocations:
        if not isinstance(alloc, mybir.MemoryLocationSet):
            continue
        name = alloc.memorylocations[0].name
        if alloc.kind == "ExternalInput":
            if name != partition_name:
                in_names.append(name)
        elif alloc.kind == "ExternalOutput":
            shape = tuple(alloc.tensor_shape)
            dtype = mybir.dt.np(alloc.dtype)
            out_names.append(name)
            out_avals.append(jax.core.ShapedArray(shape, dtype))
            zero_outs.append(_np.zeros(shape, dtype))
    n_params = len(in_names)
    n_outs = len(out_avals)
    all_in_names = list(in_names) + list(out_names)
    if partition_name is not None:
        all_in_names.append(partition_name)
    donate = tuple(range(n_params, n_params + n_outs))

    def _body(*args):
        operands = list(args)
        if partition_name is not None:
            operands.append(bass2jax.partition_id_tensor())
        outs = bass2jax._bass_exec_p.bind(
            *operands,
            out_avals=tuple(out_avals),
            in_names=tuple(all_in_names),
            out_names=tuple(out_names),
            lowering_input_output_aliases=(),
            sim_require_finite=True,
            sim_require_nnan=True,
            nc=nc,
        )
        return tuple(outs)

    devices = jax.devices()[:NCORES]
    assert len(devices) == NCORES, f"need {NCORES} cores, have {len(jax.devices())}"
    mesh = Mesh(_np.asarray(devices), ("core",))
    in_specs = (PartitionSpec("core"),) * (n_params + n_outs)
    out_specs = (PartitionSpec("core"),) * n_outs
    sharded = jax.jit(
        shard_map(_body, mesh=mesh, in_specs=in_specs, out_specs=out_specs,
                  check_rep=False),
        donate_argnums=donate, keep_unused=True)

    def run(in_maps):
        per_core = [[_np.asarray(m[name]) for name in in_names] for m in in_maps]
        concat_in = [
            _np.concatenate([per_core[c][i] for c in range(NCORES)], axis=0)
            for i in range(n_params)
        ]
        concat_zeros = [
            _np.zeros((NCORES * z.shape[0], *z.shape[1:]), z.dtype) for z in zero_outs
        ]
        out_arrs = sharded(*concat_in, *concat_zeros)
        return [
            {
                name: _np.asarray(out_arrs[i]).reshape(NCORES, *out_avals[i].shape)[c]
                for i, name in enumerate(out_names)
            }
            for c in range(NCORES)
        ]

    return run


def get_nc(n_iters=1, kb=None):
    if kb is None:
        kb = _STATE.get("kb", 6)
    key = f"nc{n_iters}_{kb}"
    if key not in _STATE:
        _STATE[key] = _build_nc(kb, n_iters)
    return _STATE[key]


def make_in_maps(queries, keys, values, valid_lens, W_q, W_k, w_v):
    queries = np.asarray(queries, dtype=np.float32)
    keys = np.asarray(keys, dtype=np.float32)
    values = np.asarray(values, dtype=np.float32)
    valid_lens = np.asarray(valid_lens)
    kb = max(1, min(KK // 128, int(-(-int(valid_lens.max()) // 128))))
    _STATE["kb"] = kb
    KBC = kb * 128
    # weights pack: W.T with output rows duplicated (128 feature rows),
    # chunked partition-major: wpk[p, c, 0:128] = wqT2 row c*128+p
    WqT2 = np.concatenate([np.asarray(W_q, np.float32).T] * 2, axis=1)  # [256,128]
    WkT2 = np.concatenate([np.asarray(W_k, np.float32).T] * 2, axis=1)
    wpk = np.concatenate([WqT2, WkT2], axis=1).astype(np.float16)      # [256,256]
    wpk = np.ascontiguousarray(wpk.reshape(2, 128, 256).transpose(1, 0, 2))
    w_v = np.asarray(w_v, dtype=np.float32)
    wv2 = np.concatenate([w_v, w_v])
    wvc = np.ascontiguousarray(wv2[:, None] * COEFFS[None, :] * QSCALE)
    in_maps = []
    for core in range(NCORES):
        b, hf = core // 2, core % 2
        L = int(valid_lens[b])
        mask = (np.arange(KBC) < L).astype(np.float32)[:, None]
        vaug = (np.concatenate([values[b, :KBC], np.ones((KBC, 1), np.float32)],
                               axis=1) * mask).astype(np.float16)
        qT = queries[b, hf * QH:(hf + 1) * QH, :].T.astype(np.float16)  # [256,QH]
        kT = keys[b, :KBC].T.astype(np.float16)                         # [256,KBC]
        in_maps.append({
            "queriesT": np.ascontiguousarray(
                qT.reshape(2, 128, QH).transpose(1, 0, 2)),
            "keysT": np.ascontiguousarray(
                kT.reshape(2, 128, KBC).transpose(1, 0, 2)),
            "vaug": np.ascontiguousarray(
                vaug.reshape(kb, 128, V + 1).transpose(1, 0, 2)),
            "wpk": wpk,
            "wvc": wvc,
        })
    return in_maps


def kernel(queries, keys, values, valid_lens, W_q, W_k, w_v):
    in_maps = make_in_maps(queries, keys, values, valid_lens, W_q, W_k, w_v)
    nc = get_nc()
    rkey = f"run_{_STATE['kb']}"
    if rkey not in _STATE:
        _STATE[rkey] = _build_runner(nc)
    results = _STATE[rkey](in_maps)
    out = np.empty((B, QFULL, V), np.float32)
    for core in range(NCORES):
        b, hf = core // 2, core % 2
        out[b, hf * QH:(hf + 1) * QH, :] = results[core]["out"].astype(np.float32)
    return out


# revision 62
# speedup vs baseline: 1.1897x; 1.1150x over previous
"""Additive-attention fused kernel for one TRN2 chip (8 NeuronCores).

Math (per batch b):
    q = queries @ W_q.T                       [Q, H]
    k = keys    @ W_k.T                       [K, H]
    scores[q,k] = sum_h w_v[h] * tanh(q[q,h] + k[k,h])
    attn = masked_softmax(scores, valid_len)  (mask: k >= L -> weight 0)
    out  = attn @ values                      [Q, V]

Sharding: data-parallel over batch B=4 x 2-way split of Q -> 8 cores,
each core handles [QH=512, :] of one batch.  No collectives needed.

v3 algorithm (v2 was a per-m DVE range-reduction + ACT Sin pipeline):

1. Sparse K: only kb = ceil(max(valid_lens)/128) k-blocks are live; the
   masked tail contributes exactly 0 through the pre-masked [values|1]
   operand, so keys/values beyond kb*128 are never shipped or touched.
2. tanh(z) ~ sum_m c_m sin(pi m z / FL) (odd Fourier series, weighted
   LSQ fit, M=8, |z| <= 2*BCLAMP), which makes scores ONE accumulated
   matmul over contraction (h, m, {sin,cos}).
3. Chebyshev recurrence for the features: with theta = 2 pi nu1 x,
       t_m = sin(m theta + phase),  t_{m+1} = 2 cos(theta) t_m - t_{m-1}
   so only m=1 needs ACT Sin (|arg| <= ~pi, where the table is exact);
   every further frequency is 2 cheap DVE ops (scalar_tensor_tensor +
   tensor_tensor) instead of range-reduction chains + Sin.
4. Hybrid fp8: frequencies m>=3 (small |c_m|) are quantized to fp8e4
   (q side pre-scaled x64) and their score contributions run as
   DoubleRow matmuls (2x PE throughput, contraction 256/instr); m=1,2
   stay fp16.  All q scales carry x64 so PSUM holds 64*scores; the exp
   reads PSUM with scale=1/64, bias=-2 (softmax shift for fp8 headroom
   elsewhere and smaller exp range).
5. Tail pipelining: score PSUM banks are split in two groups; exp of
   group A runs while PE still accumulates group B, then attn@V
   (fp16: fp8 weights/values fail the accuracy budget) accumulates
   per 128-query block with the denominator in an appended ones
   column; DVE reciprocal + gpsimd scale produce the fp16 output.

Engine budget per core (kb=6): DVE ~10us (recurrence), gpsimd ~9us
(scales/quantize/epilogue), PE ~9us (proj + scores + attn@V), ACT ~7us
(seeds + exp); HBM ~1.3MB.
"""

import math

import numpy as np

B, QFULL, KK = 4, 1024, 1024
D, H, V = 256, 64, 256
QH = 512            # Q rows per core
NCORES = 8

FM = 7              # number of frequencies
FL = 8.0            # half-period (= 2*BCLAMP so seed args fit the
                    # Sin table domain [-pi, pi] exactly)
BCLAMP = 4.0        # clamp q/k projections to +-BCLAMP
MLOW = 5            # per-bank frequency stagger: the last two k-banks use
                    # MLOW frequencies so their exp hides under the recurrence
QSCALE = 64.0       # q-feature prescale (PSUM holds QSCALE*scores)
EXP_BIAS = -2.0     # softmax shift


def _bank_M(kb):
    if kb <= 4:
        return [FM] * kb
    return [FM if j < kb - 2 else MLOW for j in range(kb)]

_STATE = {}


def _fit_coeffs():
    z = np.linspace(-2 * BCLAMP, 2 * BCLAMP, 4001)
    w = np.exp(-z ** 2 / (2 * 2.2)) + 1e-4
    A = np.sin(np.pi / FL * np.outer(z, np.arange(1, FM + 1)))
    c = np.linalg.lstsq(A * np.sqrt(w)[:, None], np.tanh(z) * np.sqrt(w),
                        rcond=None)[0]
    return c.astype(np.float32)


COEFFS = _fit_coeffs()


def _fit_cospoly():
    """Even polynomial (deg 2 in x^2) for 2*cos(2 pi nu1 x), |x| <= BCLAMP."""
    x = np.linspace(-BCLAMP, BCLAMP, 2001)
    z = x * x
    A = np.stack([np.ones_like(z), z, z * z], axis=1)
    c = np.linalg.lstsq(A, 2.0 * np.cos(np.pi / FL * x), rcond=None)[0]
    return c.astype(np.float64)


COSP = _fit_cospoly()


def _build_nc(kb, n_iters=1):
    import contextlib
    import concourse.tile as tile
    from concourse import bacc, mybir

    F32 = mybir.dt.float32
    F16 = mybir.dt.float16
    F8 = mybir.dt.float8e4
    Sin = mybir.ActivationFunctionType.Sin
    Exp = mybir.ActivationFunctionType.Exp
    AOp = mybir.AluOpType
    DR = mybir.MatmulPerfMode.DoubleRow
    TWO_PI = 2.0 * math.pi

    KBC = kb * 128                  # live key columns
    W = QH + KBC                    # feature width (q cols | k cols)
    NU1 = 1.0 / (2.0 * FL)
    BANK_M = _bank_M(kb)
    if kb <= 4:
        GROUPS = [(j0, min(j0 + 2, kb)) for j0 in range(0, kb, 2)]
    else:
        GROUPS = [(0, 2), (2, 4), (4, kb)]
    NGRP = len(GROUPS)

    nc = bacc.Bacc()
    # host layouts are partition-major so each tensor is ONE DMA:
    # qT [128, 2, QH]: [p, c, :] = queries.T fp16 rows c*128+p
    # wpk [128, 2, 256]: [p, c, 0:128] = wqT2 rows, [.., 128:256] = wkT2
    #   (w*T2 = [D, 128] with the 64 output rows duplicated -> projections
    #    emit all 128 feature partitions directly, no SBUF dup pass)
    qT_d = nc.declare_dram_parameter("queriesT", [128, 2, QH], F16, isOutput=False)
    kT_d = nc.declare_dram_parameter("keysT", [128, 2, KBC], F16, isOutput=False)
    va_d = nc.declare_dram_parameter("vaug", [128, kb, V + 1], F16, isOutput=False)
    wpk_d = nc.declare_dram_parameter("wpk", [128, 2, 256], F16, isOutput=False)
    wvc_d = nc.declare_dram_parameter("wvc", [128, FM], F32, isOutput=False)
    out_d = nc.declare_dram_parameter("out", [QH, V], F16, isOutput=True)

    with tile.TileContext(nc) as tc:
        with (
            tc.tile_pool(name="singles", bufs=1) as singles,
            tc.tile_pool(name="tpool", bufs=5) as tpool,
            tc.tile_pool(name="upool", bufs=2) as upool,
            tc.tile_pool(name="qk8", bufs=1) as qk8,
            tc.tile_pool(name="outp", bufs=2) as outp,
            # one PSUM pool; per-tag slots: score group g owns tag "scg{g}",
            # projections and attn@V accumulators reuse those banks via the
            # same tags (Tile serializes on the tag's reader/writer chain)
            tc.tile_pool(name="ps_big", bufs=1, space="PSUM") as ps_big,
            tc.For_i(0, n_iters, 1,
                     hint_engines=(mybir.EngineType.PE, mybir.EngineType.DVE,
                                   mybir.EngineType.Activation,
                                   mybir.EngineType.SP, mybir.EngineType.Pool),
                     staggered_reset=True)
            if n_iters > 1 else contextlib.nullcontext(),
        ):
            # -------- stage inputs (5 DMAs total, split across 2 queues) ----
            wpk = singles.tile([128, 2, 256], F16, tag="wpk", name="wpk")
            nc.sync.dma_start(wpk, wpk_d[:, :, :])
            qTt = singles.tile([128, 2, QH], F16, tag="qTt", name="qTt")
            nc.sync.dma_start(qTt, qT_d[:, :, :])
            kTt = singles.tile([128, 2, KBC], F16, tag="kTt", name="kTt")
            nc.sync.dma_start(kTt, kT_d[:, :, :])
            wvc_sb = singles.tile([128, FM], F32, tag="wvc", name="wvc")
            nc.sync.dma_start(wvc_sb, wvc_d[:, :])
            v_aug = singles.tile([128, kb, V + 1], F16, tag="vaug", name="vaug")
            nc.sync.dma_start(v_aug, va_d[:, :, :])

            # phase biases (radians) for the m=1 seeds
            bq = singles.tile([128, 1], F32, tag="bq", name="bq")
            nc.vector.memset(bq[0:H, :], 0.0)
            nc.vector.memset(bq[H:128, :], math.pi / 2)
            bk = singles.tile([128, 1], F32, tag="bk", name="bk")
            nc.vector.memset(bk[0:H, :], math.pi / 2)
            nc.vector.memset(bk[H:128, :], 0.0)
            bc = singles.tile([128, 1], F32, tag="bc", name="bc")
            nc.vector.memset(bc, math.pi / 2)
            be = singles.tile([128, 1], F32, tag="be", name="be")
            nc.vector.memset(be, EXP_BIAS)
            # t_0 per-partition constants (m=2 seed): q side [0;1], k side [1;0]
            t0q = singles.tile([128, 1], F32, tag="t0q", name="t0q")
            nc.vector.memset(t0q[0:H, :], 0.0)
            nc.vector.memset(t0q[H:128, :], 1.0)
            t0k = singles.tile([128, 1], F32, tag="t0k", name="t0k")
            nc.vector.memset(t0k[0:H, :], 1.0)
            nc.vector.memset(t0k[H:128, :], 0.0)
            # dummy Sin to prefetch the trig act table before xt is ready
            dum = singles.tile([128, 1], F32, tag="dum", name="dum")
            nc.scalar.activation(dum, bq[:, 0:1], Sin)

            # ------------- projections + clamp -> fp16 [128, cols] -------------
            # W rows are host-duplicated so the matmul emits all 128 feature
            # partitions; separate per-side tiles avoid false tile deps.
            # each matmul output must stay inside one 512-float PSUM bank.
            xtq = singles.tile([128, QH], F16, tag="xtq", name="xtq")
            xtk = singles.tile([128, KBC], F16, tag="xtk", name="xtk")
            nkc = -(-KBC // 512)
            psq = ps_big.tile([128, 512], F32, tag=f"scg{NGRP - 1}", name="psq")
            for c in range(2):
                nc.tensor.matmul(psq, wpk[:, c, 0:128], qTt[:, c, :],
                                 start=(c == 0), stop=(c == 1))
            nc.vector.tensor_scalar(xtq, psq, BCLAMP, -BCLAMP, AOp.min, AOp.max)
            psk = ps_big.tile([128, nkc * 512], F32, tag="scg0", name="psk")
            for ci in range(nkc):
                c0, cw = ci * 512, min(512, KBC - ci * 512)
                for c in range(2):
                    nc.tensor.matmul(psk[:, c0:c0 + cw], wpk[:, c, 128:256],
                                     kTt[:, c, c0:c0 + cw],
                                     start=(c == 0), stop=(c == 1))
            # (gpsimd cannot touch PSUM on hw; clamp-k goes to DVE)
            nc.vector.tensor_scalar(xtk, psk[:, 0:KBC], BCLAMP,
                                    -BCLAMP, AOp.min, AOp.max)

            # ------------- m=1 seeds + 2cos multiplier -------------
            # t1 = sin(2 pi nu1 x + phase); the cos rows already live inside
            # t1 (q side rows 64:128, k side rows 0:64), so c2d = 2cos comes
            # from partition-shift SBUF DMAs + one DVE scale, not two more
            # ACT Sin calls.
            t1 = singles.tile([128, W], F16, tag="t1", name="t1")
            nc.scalar.activation(t1[:, 0:QH], xtq, Sin,
                                 bias=bq[:, 0:1], scale=TWO_PI * NU1)
            nc.scalar.activation(t1[:, QH:W], xtk, Sin,
                                 bias=bk[:, 0:1], scale=TWO_PI * NU1)
            c1dup = singles.tile([128, W], F16, tag="c1dup", name="c1dup")
            nc.sync.dma_start(c1dup[0:H, 0:QH], t1[H:128, 0:QH])
            nc.sync.dma_start(c1dup[H:128, 0:QH], t1[H:128, 0:QH])
            nc.sync.dma_start(c1dup[0:H, QH:W], t1[0:H, QH:W])
            nc.sync.dma_start(c1dup[H:128, QH:W], t1[0:H, QH:W])
            c2d = singles.tile([128, W], F16, tag="c2d", name="c2d")
            nc.vector.tensor_scalar(c2d, c1dup, 2.0, None, AOp.mult)

            # ------------- features: Chebyshev recurrence + scales -------------
            # per m: u = t_{m-1} . c2d ; t_m = u - t_{m-2}   (DVE, all fp16)
            # plus qf_m = t_m[:, :QH] * (w_v c_m 64) for the score matmuls.
            # 3-level per-bank frequency stagger: each group's exp fires as
            # soon as its banks close, hiding under the rest of the recurrence
            bank_M = BANK_M
            qf16 = {}
            tms = {1: t1}
            for m in range(1, FM + 1):
                if m >= 2:
                    u = upool.tile([128, W], F16, tag="u", name=f"u{m}")
                    nc.vector.tensor_tensor(u, tms[m - 1], c2d, AOp.mult)
                    # every t_m is a PE matmul operand later; keep them all
                    tm = singles.tile([128, W], F16, tag=f"t{m}", name=f"t{m}")
                    if m == 2:
                        # t2 = u - t0 with t0 constant per partition-half
                        nc.vector.tensor_scalar(tm[:, 0:QH], u[:, 0:QH],
                                                t0q[:, 0:1], None, AOp.subtract)
                        nc.vector.tensor_scalar(tm[:, QH:W], u[:, QH:W],
                                                t0k[:, 0:1], None, AOp.subtract)
                    else:
                        nc.vector.tensor_tensor(tm, u, tms[m - 2], AOp.subtract)
                    tms[m] = tm
                tm = tms[m]
                qf = singles.tile([128, QH], F16, tag=f"qf{m}", name=f"qf{m}")
                nc.vector.tensor_scalar(qf, tm[:, 0:QH],
                                        wvc_sb[:, m - 1:m], None, AOp.mult)
                qf16[m] = qf

            # ------------- scores (PSUM accumulate), staggered tail -------------
            # banks grouped into per-tag PSUM tiles: full-M banks in pairs,
            # the LOWB low-M banks as the last group.  m-major emission =
            # matmuls fire the moment qf_m lands; low banks close at MLOW and
            # their exp hides under the rest of the recurrence.
            groups = GROUPS
            scg = []
            for g, (j0, j1) in enumerate(groups):
                scg.append(ps_big.tile([128, j1 - j0, QH], F32, tag=f"scg{g}",
                                       name=f"scg{g}"))
            jmap = {}
            group_M = []
            for g, (j0, j1) in enumerate(groups):
                for j in range(j0, j1):
                    jmap[j] = (g, j - j0)
                group_M.append(max(bank_M[j] for j in range(j0, j1)))
            p16 = [None] * NGRP
            for m in range(1, FM + 1):
                for j in range(kb):
                    if m > bank_M[j]:
                        continue
                    g, ji = jmap[j]
                    nc.tensor.matmul(
                        scg[g][:, ji, :],
                        tms[m][:, QH + j * 128:QH + (j + 1) * 128],
                        qf16[m], start=(m == 1), stop=(m == bank_M[j]))
                # emit each group's exp right after its last m-sweep so it
                # runs during the remaining recurrence
                for g, (j0, j1) in enumerate(groups):
                    if group_M[g] != m:
                        continue
                    pg = singles.tile([128, j1 - j0, QH], F16, tag=f"p16_{g}",
                                      name=f"p16_{g}")
                    nc.scalar.activation(pg.rearrange("p a b -> p (a b)"),
                                         scg[g].rearrange("p a b -> p (a b)"),
                                         Exp, bias=be[:, 0:1],
                                         scale=1.0 / QSCALE)
                    p16[g] = pg

            # ------------- attn @ [values | ones] + epilogue -------------
            # 4 concurrent accumulators: first pair in the last group's
            # banks (freed by the earliest exp), second in the mid group's;
            # j-major emission, latest-closing banks last
            av01 = ps_big.tile([128, 2, 512], F32, tag=f"scg{NGRP - 1}",
                               name="av01")
            av23 = ps_big.tile([128, 2, 512], F32,
                               tag=f"scg{1 if NGRP >= 2 else 0}", name="av23")
            avs = [av01[:, 0, 0:V + 1], av01[:, 1, 0:V + 1],
                   av23[:, 0, 0:V + 1], av23[:, 1, 0:V + 1]]
            jorder = sorted(range(kb), key=lambda j: (bank_M[j], -j))
            for idx, j in enumerate(jorder):
                g, ji = jmap[j]
                pj = p16[g][:, ji, :]
                for qb in range(QH // 128):
                    nc.tensor.matmul(avs[qb], pj[:, qb * 128:(qb + 1) * 128],
                                     v_aug[:, j, :], start=(idx == 0),
                                     stop=(idx == kb - 1))
            # output in two half-tiles so the first DMA overlaps the rest;
            # epilogue on DVE (reciprocal + scale), ACT is done after exps
            o16h = [outp.tile([128, 2, V], F16, tag=f"o16_{h}", name=f"o16_{h}")
                    for h in range(2)]
            for qb in range(QH // 128):
                av = avs[qb]
                rcp = outp.tile([128, 1], F32, tag="rcp", name=f"rcp{qb}")
                nc.vector.reciprocal(rcp, av[:, V:V + 1])
                nc.vector.tensor_scalar(o16h[qb // 2][:, qb % 2, :], av[:, 0:V],
                                        rcp[:, 0:1], None, AOp.mult)
                if qb % 2 == 1:
                    nc.sync.dma_start(
                        out_d.rearrange("(a p) v -> p a v", p=128)
                        [:, (qb - 1):(qb + 1), :], o16h[qb // 2])

    nc.finalize()
    return nc


def _build_runner(nc):
    """Cached multi-core PJRT runner (keeps the jitted callable so repeat
    calls don't retrace/recompile)."""
    import jax
    import numpy as _np
    from jax.sharding import Mesh, PartitionSpec
    from jax.experimental.shard_map import shard_map
    from concourse import bass2jax, mybir

    bass2jax.install_neuronx_cc_hook()

    partition_name = nc.partition_id_tensor.name if nc.partition_id_tensor else None
    in_names, out_names, out_avals, zero_outs = [], [], [], []
    for alloc in nc.m.functions[0].allocations:
        if not isinstance(alloc, mybir.MemoryLocationSet):
            continue
        name = alloc.memorylocations[0].name
        if alloc.kind == "ExternalInput":
            if name != partition_name:
                in_names.append(name)
        elif alloc.kind == "ExternalOutput":
            shape = tuple(alloc.tensor_shape)
            dtype = mybir.dt.np(alloc.dtype)
            out_names.append(name)
            out_avals.append(jax.core.ShapedArray(shape, dtype))
            zero_outs.append(_np.zeros(shape, dtype))
    n_params = len(in_names)
    n_outs = len(out_avals)
    all_in_names = list(in_names) + list(out_names)
    if partition_name is not None:
        all_in_names.append(partition_name)
    donate = tuple(range(n_params, n_params + n_outs))

    def _body(*args):
        operands = list(args)
        if partition_name is not None:
            operands.append(bass2jax.partition_id_tensor())
        outs = bass2jax._bass_exec_p.bind(
            *operands,
            out_avals=tuple(out_avals),
            in_names=tuple(all_in_names),
            out_names=tuple(out_names),
            lowering_input_output_aliases=(),
            sim_require_finite=True,
            sim_require_nnan=True,
            nc=nc,
        )
        return tuple(outs)

    devices = jax.devices()[:NCORES]
    assert len(devices) == NCORES, f"need {NCORES} cores, have {len(jax.devices())}"
    mesh = Mesh(_np.asarray(devices), ("core",))
    in_specs = (PartitionSpec("core"),) * (n_params + n_outs)
    out_specs = (PartitionSpec("core"),) * n_outs
    sharded = jax.jit(
        shard_map(_body, mesh=mesh, in_specs=in_specs, out_specs=out_specs,
                  check_rep=False),
        donate_argnums=donate, keep_unused=True)

    def run(in_maps):
        per_core = [[_np.asarray(m[name]) for name in in_names] for m in in_maps]
        concat_in = [
            _np.concatenate([per_core[c][i] for c in range(NCORES)], axis=0)
            for i in range(n_params)
        ]
        concat_zeros = [
            _np.zeros((NCORES * z.shape[0], *z.shape[1:]), z.dtype) for z in zero_outs
        ]
        out_arrs = sharded(*concat_in, *concat_zeros)
        return [
            {
                name: _np.asarray(out_arrs[i]).reshape(NCORES, *out_avals[i].shape)[c]
                for i, name in enumerate(out_names)
            }
            for c in range(NCORES)
        ]

    return run


def get_nc(n_iters=1, kb=None):
    if kb is None:
        kb = _STATE.get("kb", 6)
    key = f"nc{n_iters}_{kb}"
    if key not in _STATE:
        _STATE[key] = _build_nc(kb, n_iters)
    return _STATE[key]


def make_in_maps(queries, keys, values, valid_lens, W_q, W_k, w_v):
    queries = np.asarray(queries, dtype=np.float32)
    keys = np.asarray(keys, dtype=np.float32)
    values = np.asarray(values, dtype=np.float32)
    valid_lens = np.asarray(valid_lens)
    kb = max(1, min(KK // 128, int(-(-int(valid_lens.max()) // 128))))
    _STATE["kb"] = kb
    KBC = kb * 128
    # weights pack: W.T with output rows duplicated (128 feature rows),
    # chunked partition-major: wpk[p, c, 0:128] = wqT2 row c*128+p
    WqT2 = np.concatenate([np.asarray(W_q, np.float32).T] * 2, axis=1)  # [256,128]
    WkT2 = np.concatenate([np.asarray(W_k, np.float32).T] * 2, axis=1)
    wpk = np.concatenate([WqT2, WkT2], axis=1).astype(np.float16)      # [256,256]
    wpk = np.ascontiguousarray(wpk.reshape(2, 128, 256).transpose(1, 0, 2))
    w_v = np.asarray(w_v, dtype=np.float32)
    wv2 = np.concatenate([w_v, w_v])
    wvc = np.ascontiguousarray(wv2[:, None] * COEFFS[None, :] * QSCALE)
    in_maps = []
    for core in range(NCORES):
        b, hf = core // 2, core % 2
        L = int(valid_lens[b])
        mask = (np.arange(KBC) < L).astype(np.float32)[:, None]
        vaug = (np.concatenate([values[b, :KBC], np.ones((KBC, 1), np.float32)],
                               axis=1) * mask).astype(np.float16)
        qT = queries[b, hf * QH:(hf + 1) * QH, :].T.astype(np.float16)  # [256,QH]
        kT = keys[b, :KBC].T.astype(np.float16)                         # [256,KBC]
        in_maps.append({
            "queriesT": np.ascontiguousarray(
                qT.reshape(2, 128, QH).transpose(1, 0, 2)),
            "keysT": np.ascontiguousarray(
                kT.reshape(2, 128, KBC).transpose(1, 0, 2)),
            "vaug": np.ascontiguousarray(
                vaug.reshape(kb, 128, V + 1).transpose(1, 0, 2)),
            "wpk": wpk,
            "wvc": wvc,
        })
    return in_maps


def kernel(queries, keys, values, valid_lens, W_q, W_k, w_v):
    in_maps = make_in_maps(queries, keys, values, valid_lens, W_q, W_k, w_v)
    nc = get_nc()
    rkey = f"run_{_STATE['kb']}"
    if rkey not in _STATE:
        _STATE[rkey] = _build_runner(nc)
    results = _STATE[rkey](in_maps)
    out = np.empty((B, QFULL, V), np.float32)
    for core in range(NCORES):
        b, hf = core // 2, core % 2
        out[b, hf * QH:(hf + 1) * QH, :] = results[core]["out"].astype(np.float32)
    return out


# revision 64
# speedup vs baseline: 1.2984x; 1.0913x over previous
"""Additive-attention fused kernel for one TRN2 chip (8 NeuronCores).

Math (per batch b):
    q = queries @ W_q.T                       [Q, H]
    k = keys    @ W_k.T                       [K, H]
    scores[q,k] = sum_h w_v[h] * tanh(q[q,h] + k[k,h])
    attn = masked_softmax(scores, valid_len)  (mask: k >= L -> weight 0)
    out  = attn @ values                      [Q, V]

Sharding: data-parallel over batch B=4 x 2-way split of Q -> 8 cores,
each core handles [QH=512, :] of one batch.  No collectives needed.

Algorithm (HW-calibrated: ACT table ops ~2.4ns/col, DVE fp16 ~0.3ns/col,
GPSIMD unusable at ~10us/op, so everything elementwise lives on DVE):

1. Sparse K: only kb = ceil(max(valid_lens)/128) k-blocks are live; the
   masked tail contributes exactly 0 through the pre-masked [values|1]
   operand, so keys/values beyond kb*128 are never shipped or touched.
2. tanh(z) ~ sum_m c_m sin(pi m z / FL) (odd Fourier series, weighted
   LSQ fit, M=FM, |z| <= 2*BCLAMP), which makes scores an accumulated
   matmul over contraction (h, m, {sin,cos}) -- 128 partitions per m.
3. Chebyshev recurrence for the features: with theta = 2 pi nu1 x,
       t_m = sin(m theta + phase),  t_{m+1} = 2 cos(theta) t_m - t_{m-1}
   so only m=1 needs ACT Sin (args within the table's exact [-pi, pi]);
   every further frequency is 2 fast DVE tensor_tensor ops.  The 2cos
   multiplier is assembled from t1's own cos rows via partition-shift
   SBUF DMAs (no extra ACT calls).  W rows are host-duplicated so the
   projections emit all 128 feature partitions directly.
4. Per-bank frequency stagger: the last two k-banks stop at MLOW
   frequencies, so their exp runs on ACT while DVE still computes the
   remaining recurrence steps; only the full-M groups' exps are in the
   tail.  m-major matmul emission = every matmul fires the moment its
   qf_m lands.
5. attn@V (all fp16; fp8 weights/values fail the accuracy budget)
   accumulates per 128-query block with the denominator in an appended
   ones column, j-major across 4 concurrent PSUM accumulators living in
   banks freed by the earliest exps; DVE reciprocal + scale produce the
   fp16 output, DMA'd out in two halves.

Engine budget per core (kb=6): DVE ~11us (recurrence + scales + epilogue),
ACT ~12us (seeds + exps, mostly overlapped), PE ~12us (proj + scores +
attn@V), gpsimd unused; HBM ~1.3MB.  HW: ~34.5us/call, rel err 7.5e-3.
"""

import math

import numpy as np

B, QFULL, KK = 4, 1024, 1024
D, H, V = 256, 64, 256
QH = 512            # Q rows per core
NCORES = 8

FM = 7              # number of frequencies
FL = 8.0            # half-period (= 2*BCLAMP so seed args fit the
                    # Sin table domain [-pi, pi] exactly)
BCLAMP = 4.0        # clamp q/k projections to +-BCLAMP
MLOW = 5            # per-bank frequency stagger: the last two k-banks use
                    # MLOW frequencies so their exp hides under the recurrence
QSCALE = 64.0       # q-feature prescale (PSUM holds QSCALE*scores)
EXP_BIAS = -2.0     # softmax shift


def _bank_M(kb):
    if kb <= 4:
        return [FM] * kb
    return [FM if j < kb - 2 else MLOW for j in range(kb)]

_STATE = {}


def _fit_coeffs():
    z = np.linspace(-2 * BCLAMP, 2 * BCLAMP, 4001)
    w = np.exp(-z ** 2 / (2 * 2.2)) + 1e-4
    A = np.sin(np.pi / FL * np.outer(z, np.arange(1, FM + 1)))
    c = np.linalg.lstsq(A * np.sqrt(w)[:, None], np.tanh(z) * np.sqrt(w),
                        rcond=None)[0]
    return c.astype(np.float32)


COEFFS = _fit_coeffs()


def _fit_cospoly():
    """Even polynomial (deg 2 in x^2) for 2*cos(2 pi nu1 x), |x| <= BCLAMP."""
    x = np.linspace(-BCLAMP, BCLAMP, 2001)
    z = x * x
    A = np.stack([np.ones_like(z), z, z * z], axis=1)
    c = np.linalg.lstsq(A, 2.0 * np.cos(np.pi / FL * x), rcond=None)[0]
    return c.astype(np.float64)


COSP = _fit_cospoly()


def _build_nc(kb, n_iters=1):
    import contextlib
    import concourse.tile as tile
    from concourse import bacc, mybir

    F32 = mybir.dt.float32
    F16 = mybir.dt.float16
    F8 = mybir.dt.float8e4
    Sin = mybir.ActivationFunctionType.Sin
    Exp = mybir.ActivationFunctionType.Exp
    AOp = mybir.AluOpType
    DR = mybir.MatmulPerfMode.DoubleRow
    TWO_PI = 2.0 * math.pi

    KBC = kb * 128                  # live key columns
    W = QH + KBC                    # feature width (q cols | k cols)
    NU1 = 1.0 / (2.0 * FL)
    BANK_M = _bank_M(kb)
    if kb <= 4:
        GROUPS = [(j0, min(j0 + 2, kb)) for j0 in range(0, kb, 2)]
    else:
        # full-M banks in two groups, the MLOW banks as their own group so
        # their exp is not held back by a full-M bank
        GROUPS = [(0, 2), (2, kb - 2), (kb - 2, kb)]
    NGRP = len(GROUPS)

    nc = bacc.Bacc()
    # host layouts are partition-major so each tensor is ONE DMA:
    # qT [128, 2, QH]: [p, c, :] = queries.T fp16 rows c*128+p
    # wpk [128, 2, 256]: [p, c, 0:128] = wqT2 rows, [.., 128:256] = wkT2
    #   (w*T2 = [D, 128] with the 64 output rows duplicated -> projections
    #    emit all 128 feature partitions directly, no SBUF dup pass)
    qT_d = nc.declare_dram_parameter("queriesT", [128, 2, QH], F16, isOutput=False)
    kT_d = nc.declare_dram_parameter("keysT", [128, 2, KBC], F16, isOutput=False)
    va_d = nc.declare_dram_parameter("vaug", [128, kb, V + 1], F16, isOutput=False)
    wpk_d = nc.declare_dram_parameter("wpk", [128, 2, 256], F16, isOutput=False)
    wvc_d = nc.declare_dram_parameter("wvc", [128, FM], F32, isOutput=False)
    out_d = nc.declare_dram_parameter("out", [QH, V], F16, isOutput=True)

    with tile.TileContext(nc) as tc:
        with (
            tc.tile_pool(name="singles", bufs=1) as singles,
            tc.tile_pool(name="tpool", bufs=5) as tpool,
            tc.tile_pool(name="upool", bufs=2) as upool,
            tc.tile_pool(name="qk8", bufs=1) as qk8,
            tc.tile_pool(name="outp", bufs=2) as outp,
            # one PSUM pool; per-tag slots: score group g owns tag "scg{g}",
            # projections and attn@V accumulators reuse those banks via the
            # same tags (Tile serializes on the tag's reader/writer chain)
            tc.tile_pool(name="ps_big", bufs=1, space="PSUM") as ps_big,
            tc.For_i(0, n_iters, 1,
                     hint_engines=(mybir.EngineType.PE, mybir.EngineType.DVE,
                                   mybir.EngineType.Activation,
                                   mybir.EngineType.SP, mybir.EngineType.Pool),
                     staggered_reset=True)
            if n_iters > 1 else contextlib.nullcontext(),
        ):
            # -------- stage inputs (5 DMAs total, split across 2 queues) ----
            wpk = singles.tile([128, 2, 256], F16, tag="wpk", name="wpk")
            nc.sync.dma_start(wpk, wpk_d[:, :, :])
            qTt = singles.tile([128, 2, QH], F16, tag="qTt", name="qTt")
            nc.sync.dma_start(qTt, qT_d[:, :, :])
            kTt = singles.tile([128, 2, KBC], F16, tag="kTt", name="kTt")
            nc.sync.dma_start(kTt, kT_d[:, :, :])
            wvc_sb = singles.tile([128, FM], F32, tag="wvc", name="wvc")
            nc.sync.dma_start(wvc_sb, wvc_d[:, :])
            v_aug = singles.tile([128, kb, V + 1], F16, tag="vaug", name="vaug")
            nc.sync.dma_start(v_aug, va_d[:, :, :])

            # phase biases (radians) for the m=1 seeds
            bq = singles.tile([128, 1], F32, tag="bq", name="bq")
            nc.vector.memset(bq[0:H, :], 0.0)
            nc.vector.memset(bq[H:128, :], math.pi / 2)
            bk = singles.tile([128, 1], F32, tag="bk", name="bk")
            nc.vector.memset(bk[0:H, :], math.pi / 2)
            nc.vector.memset(bk[H:128, :], 0.0)
            bc = singles.tile([128, 1], F32, tag="bc", name="bc")
            nc.vector.memset(bc, math.pi / 2)
            be = singles.tile([128, 1], F32, tag="be", name="be")
            nc.vector.memset(be, EXP_BIAS)
            # t_0 per-partition constants (m=2 seed): q side [0;1], k side [1;0]
            t0q = singles.tile([128, 1], F32, tag="t0q", name="t0q")
            nc.vector.memset(t0q[0:H, :], 0.0)
            nc.vector.memset(t0q[H:128, :], 1.0)
            t0k = singles.tile([128, 1], F32, tag="t0k", name="t0k")
            nc.vector.memset(t0k[0:H, :], 1.0)
            nc.vector.memset(t0k[H:128, :], 0.0)
            # dummy Sin to prefetch the trig act table before xt is ready
            dum = singles.tile([128, 1], F32, tag="dum", name="dum")
            nc.scalar.activation(dum, bq[:, 0:1], Sin)

            # ------------- projections + clamp -> fp16 [128, cols] -------------
            # W rows are host-duplicated so the matmul emits all 128 feature
            # partitions; separate per-side tiles avoid false tile deps.
            # each matmul output must stay inside one 512-float PSUM bank.
            xtq = singles.tile([128, QH], F16, tag="xtq", name="xtq")
            xtk = singles.tile([128, KBC], F16, tag="xtk", name="xtk")
            nkc = -(-KBC // 512)
            psq = ps_big.tile([128, 512], F32, tag=f"scg{NGRP - 1}", name="psq")
            for c in range(2):
                nc.tensor.matmul(psq, wpk[:, c, 0:128], qTt[:, c, :],
                                 start=(c == 0), stop=(c == 1))
            nc.vector.tensor_scalar(xtq, psq, BCLAMP, -BCLAMP, AOp.min, AOp.max)
            psk = ps_big.tile([128, nkc * 512], F32, tag="scg0", name="psk")
            for ci in range(nkc):
                c0, cw = ci * 512, min(512, KBC - ci * 512)
                for c in range(2):
                    nc.tensor.matmul(psk[:, c0:c0 + cw], wpk[:, c, 128:256],
                                     kTt[:, c, c0:c0 + cw],
                                     start=(c == 0), stop=(c == 1))
            # (gpsimd cannot touch PSUM on hw; clamp-k goes to DVE)
            nc.vector.tensor_scalar(xtk, psk[:, 0:KBC], BCLAMP,
                                    -BCLAMP, AOp.min, AOp.max)

            # ------------- m=1 seeds + 2cos multiplier -------------
            # t1 = sin(2 pi nu1 x + phase); the cos rows already live inside
            # t1 (q side rows 64:128, k side rows 0:64), so c2d = 2cos comes
            # from partition-shift SBUF DMAs + one DVE scale, not two more
            # ACT Sin calls.
            t1 = singles.tile([128, W], F16, tag="t1", name="t1")
            nc.scalar.activation(t1[:, 0:QH], xtq, Sin,
                                 bias=bq[:, 0:1], scale=TWO_PI * NU1)
            nc.scalar.activation(t1[:, QH:W], xtk, Sin,
                                 bias=bk[:, 0:1], scale=TWO_PI * NU1)
            c1dup = singles.tile([128, W], F16, tag="c1dup", name="c1dup")
            nc.sync.dma_start(c1dup[0:H, 0:QH], t1[H:128, 0:QH])
            nc.sync.dma_start(c1dup[H:128, 0:QH], t1[H:128, 0:QH])
            nc.sync.dma_start(c1dup[0:H, QH:W], t1[0:H, QH:W])
            nc.sync.dma_start(c1dup[H:128, QH:W], t1[0:H, QH:W])
            c2d = singles.tile([128, W], F16, tag="c2d", name="c2d")
            nc.vector.tensor_scalar(c2d, c1dup, 2.0, None, AOp.mult)

            # ------------- features: Chebyshev recurrence + scales -------------
            # per m: u = t_{m-1} . c2d ; t_m = u - t_{m-2}   (DVE, all fp16)
            # plus qf_m = t_m[:, :QH] * (w_v c_m 64) for the score matmuls.
            # 3-level per-bank frequency stagger: each group's exp fires as
            # soon as its banks close, hiding under the rest of the recurrence
            bank_M = BANK_M
            qf16 = {}
            tms = {1: t1}
            for m in range(1, FM + 1):
                if m >= 2:
                    u = upool.tile([128, W], F16, tag="u", name=f"u{m}")
                    nc.vector.tensor_tensor(u, tms[m - 1], c2d, AOp.mult)
                    # every t_m is a PE matmul operand later; keep them all
                    tm = singles.tile([128, W], F16, tag=f"t{m}", name=f"t{m}")
                    if m == 2:
                        # t2 = u - t0 with t0 constant per partition-half
                        nc.vector.tensor_scalar(tm[:, 0:QH], u[:, 0:QH],
                                                t0q[:, 0:1], None, AOp.subtract)
                        nc.vector.tensor_scalar(tm[:, QH:W], u[:, QH:W],
                                                t0k[:, 0:1], None, AOp.subtract)
                    else:
                        nc.vector.tensor_tensor(tm, u, tms[m - 2], AOp.subtract)
                    tms[m] = tm
                tm = tms[m]
                qf = singles.tile([128, QH], F16, tag=f"qf{m}", name=f"qf{m}")
                nc.vector.tensor_scalar(qf, tm[:, 0:QH],
                                        wvc_sb[:, m - 1:m], None, AOp.mult)
                qf16[m] = qf

            # ------------- scores (PSUM accumulate), staggered tail -------------
            # banks grouped into per-tag PSUM tiles: full-M banks in pairs,
            # the LOWB low-M banks as the last group.  m-major emission =
            # matmuls fire the moment qf_m lands; low banks close at MLOW and
            # their exp hides under the rest of the recurrence.
            groups = GROUPS
            scg = []
            for g, (j0, j1) in enumerate(groups):
                scg.append(ps_big.tile([128, j1 - j0, QH], F32, tag=f"scg{g}",
                                       name=f"scg{g}"))
            jmap = {}
            group_M = []
            for g, (j0, j1) in enumerate(groups):
                for j in range(j0, j1):
                    jmap[j] = (g, j - j0)
                group_M.append(max(bank_M[j] for j in range(j0, j1)))
            p16 = [None] * NGRP
            for m in range(1, FM + 1):
                for j in range(kb):
                    if m > bank_M[j]:
                        continue
                    g, ji = jmap[j]
                    nc.tensor.matmul(
                        scg[g][:, ji, :],
                        tms[m][:, QH + j * 128:QH + (j + 1) * 128],
                        qf16[m], start=(m == 1), stop=(m == bank_M[j]))
                # emit each group's exp right after its last m-sweep so it
                # runs during the remaining recurrence
                for g, (j0, j1) in enumerate(groups):
                    if group_M[g] != m:
                        continue
                    pg = singles.tile([128, j1 - j0, QH], F16, tag=f"p16_{g}",
                                      name=f"p16_{g}")
                    nc.scalar.activation(pg.rearrange("p a b -> p (a b)"),
                                         scg[g].rearrange("p a b -> p (a b)"),
                                         Exp, bias=be[:, 0:1],
                                         scale=1.0 / QSCALE)
                    p16[g] = pg

            # ------------- attn @ [values | ones] + epilogue -------------
            # 4 concurrent accumulators: first pair in the last group's
            # banks (freed by the earliest exp), second in the mid group's;
            # j-major emission, latest-closing banks last
            av01 = ps_big.tile([128, 2, 512], F32, tag=f"scg{NGRP - 1}",
                               name="av01")
            av23 = ps_big.tile([128, 2, 512], F32,
                               tag=f"scg{1 if NGRP >= 2 else 0}", name="av23")
            avs = [av01[:, 0, 0:V + 1], av01[:, 1, 0:V + 1],
                   av23[:, 0, 0:V + 1], av23[:, 1, 0:V + 1]]
            jorder = sorted(range(kb), key=lambda j: (bank_M[j], -j))
            for idx, j in enumerate(jorder):
                g, ji = jmap[j]
                pj = p16[g][:, ji, :]
                for qb in range(QH // 128):
                    nc.tensor.matmul(avs[qb], pj[:, qb * 128:(qb + 1) * 128],
                                     v_aug[:, j, :], start=(idx == 0),
                                     stop=(idx == kb - 1))
            # output in two half-tiles so the first DMA overlaps the rest;
            # epilogue on DVE (reciprocal + scale), ACT is done after exps
            o16h = [outp.tile([128, 2, V], F16, tag=f"o16_{h}", name=f"o16_{h}")
                    for h in range(2)]
            for qb in range(QH // 128):
                av = avs[qb]
                rcp = outp.tile([128, 1], F32, tag="rcp", name=f"rcp{qb}")
                nc.vector.reciprocal(rcp, av[:, V:V + 1])
                nc.vector.tensor_scalar(o16h[qb // 2][:, qb % 2, :], av[:, 0:V],
                                        rcp[:, 0:1], None, AOp.mult)
                if qb % 2 == 1:
                    nc.sync.dma_start(
                        out_d.rearrange("(a p) v -> p a v", p=128)
                        [:, (qb - 1):(qb + 1), :], o16h[qb // 2])

    nc.finalize()
    return nc


def _build_runner(nc):
    """Cached multi-core PJRT runner (keeps the jitted callable so repeat
    calls don't retrace/recompile)."""
    import jax
    import numpy as _np
    from jax.sharding import Mesh, PartitionSpec
    from jax.experimental.shard_map import shard_map
    from concourse import bass2jax, mybir

    bass2jax.install_neuronx_cc_hook()

    partition_name = nc.partition_id_tensor.name if nc.partition_id_tensor else None
    in_names, out_names, out_avals, zero_outs = [], [], [], []
    for alloc in nc.m.functions[0].allocations:
        if not isinstance(alloc, mybir.MemoryLocationSet):
            continue
        name = alloc.memorylocations[0].name
        if alloc.kind == "ExternalInput":
            if name != partition_name:
                in_names.append(name)
        elif alloc.kind == "ExternalOutput":
            shape = tuple(alloc.tensor_shape)
            dtype = mybir.dt.np(alloc.dtype)
            out_names.append(name)
            out_avals.append(jax.core.ShapedArray(shape, dtype))
            zero_outs.append(_np.zeros(shape, dtype))
    n_params = len(in_names)
    n_outs = len(out_avals)
    all_in_names = list(in_names) + list(out_names)
    if partition_name is not None:
        all_in_names.append(partition_name)
    donate = tuple(range(n_params, n_params + n_outs))

    def _body(*args):
        operands = list(args)
        if partition_name is not None:
            operands.append(bass2jax.partition_id_tensor())
        outs = bass2jax._bass_exec_p.bind(
            *operands,
            out_avals=tuple(out_avals),
            in_names=tuple(all_in_names),
            out_names=tuple(out_names),
            lowering_input_output_aliases=(),
            sim_require_finite=True,
            sim_require_nnan=True,
            nc=nc,
        )
        return tuple(outs)

    devices = jax.devices()[:NCORES]
    assert len(devices) == NCORES, f"need {NCORES} cores, have {len(jax.devices())}"
    mesh = Mesh(_np.asarray(devices), ("core",))
    in_specs = (PartitionSpec("core"),) * (n_params + n_outs)
    out_specs = (PartitionSpec("core"),) * n_outs
    sharded = jax.jit(
        shard_map(_body, mesh=mesh, in_specs=in_specs, out_specs=out_specs,
                  check_rep=False),
        donate_argnums=donate, keep_unused=True)

    def run(in_maps):
        per_core = [[_np.asarray(m[name]) for name in in_names] for m in in_maps]
        concat_in = [
            _np.concatenate([per_core[c][i] for c in range(NCORES)], axis=0)
            for i in range(n_params)
        ]
        concat_zeros = [
            _np.zeros((NCORES * z.shape[0], *z.shape[1:]), z.dtype) for z in zero_outs
        ]
        out_arrs = sharded(*concat_in, *concat_zeros)
        return [
            {
                name: _np.asarray(out_arrs[i]).reshape(NCORES, *out_avals[i].shape)[c]
                for i, name in enumerate(out_names)
            }
            for c in range(NCORES)
        ]

    return run


def get_nc(n_iters=1, kb=None):
    if kb is None:
        kb = _STATE.get("kb", 6)
    key = f"nc{n_iters}_{kb}"
    if key not in _STATE:
        _STATE[key] = _build_nc(kb, n_iters)
    return _STATE[key]


def make_in_maps(queries, keys, values, valid_lens, W_q, W_k, w_v):
    queries = np.asarray(queries, dtype=np.float32)
    keys = np.asarray(keys, dtype=np.float32)
    values = np.asarray(values, dtype=np.float32)
    valid_lens = np.asarray(valid_lens)
    kb = max(1, min(KK // 128, int(-(-int(valid_lens.max()) // 128))))
    _STATE["kb"] = kb
    KBC = kb * 128
    # weights pack: W.T with output rows duplicated (128 feature rows),
    # chunked partition-major: wpk[p, c, 0:128] = wqT2 row c*128+p
    WqT2 = np.concatenate([np.asarray(W_q, np.float32).T] * 2, axis=1)  # [256,128]
    WkT2 = np.concatenate([np.asarray(W_k, np.float32).T] * 2, axis=1)
    wpk = np.concatenate([WqT2, WkT2], axis=1).astype(np.float16)      # [256,256]
    wpk = np.ascontiguousarray(wpk.reshape(2, 128, 256).transpose(1, 0, 2))
    w_v = np.asarray(w_v, dtype=np.float32)
    wv2 = np.concatenate([w_v, w_v])
    wvc = np.ascontiguousarray(wv2[:, None] * COEFFS[None, :] * QSCALE)
    in_maps = []
    for core in range(NCORES):
        b, hf = core // 2, core % 2
        L = int(valid_lens[b])
        mask = (np.arange(KBC) < L).astype(np.float32)[:, None]
        vaug = (np.concatenate([values[b, :KBC], np.ones((KBC, 1), np.float32)],
                               axis=1) * mask).astype(np.float16)
        qT = queries[b, hf * QH:(hf + 1) * QH, :].T.astype(np.float16)  # [256,QH]
        kT = keys[b, :KBC].T.astype(np.float16)                         # [256,KBC]
        in_maps.append({
            "queriesT": np.ascontiguousarray(
                qT.reshape(2, 128, QH).transpose(1, 0, 2)),
            "keysT": np.ascontiguousarray(
                kT.reshape(2, 128, KBC).transpose(1, 0, 2)),
            "vaug": np.ascontiguousarray(
                vaug.reshape(kb, 128, V + 1).transpose(1, 0, 2)),
            "wpk": wpk,
            "wvc": wvc,
        })
    return in_maps


def kernel(queries, keys, values, valid_lens, W_q, W_k, w_v):
    in_maps = make_in_maps(queries, keys, values, valid_lens, W_q, W_k, w_v)
    nc = get_nc()
    rkey = f"run_{_STATE['kb']}"
    if rkey not in _STATE:
        _STATE[rkey] = _build_runner(nc)
    results = _STATE[rkey](in_maps)
    out = np.empty((B, QFULL, V), np.float32)
    for core in range(NCORES):
        b, hf = core // 2, core % 2
        out[b, hf * QH:(hf + 1) * QH, :] = results[core]["out"].astype(np.float32)
    return out
